# revision 61
# baseline (speedup 1.0000x reference)
"""Trainium2 Bass kernel for nn_Diagnet (S=1024, B=64, I=512, H=2048, O=512).

    u = einsum('sbi,hi->sbh', X, W_ih)
    h_t = |u_t + hh * h_{t-1}|   (scan over S, only final h needed)
    Y = h_final @ W_ho.T + b_ho

Strategy (8 NeuronCores, data-parallel over batch, BC=8 rows per core):

* H lanes are permuted so hh is sorted descending and split into 16
  chunks of 128.  A chunk whose largest decay a satisfies a^K < tol
  only needs the last K steps, so each chunk gets a window K_g and the
  GEMM + scan skip everything earlier.  Chunk 0 keeps the full 1024.
* The recurrence is a custom DVE instruction that folds a window in
  one go: m[t] = |m[t-1] - u[t]*scn[t]| with scn[t] = -a^(K-1-t)
  (prescale folds the decay into the stream; the minus sign turns
  ABSOLUTE_DIFF into abs-add).  h_final = last element.
* All 8 batch rows fold in ONE scan instruction per segment via a
  3-column header per row: a BIG separator pair scaled (-1, +1)
  absorbs and exactly zeroes the running state between rows, then a
  seed column (scn=-1) re-injects that row's carry from the previous
  segment (h >= 0 so |0 - s*(-1)| = s).  Seed values are copied
  between segments by a tiny DVE copy, keeping the serial chain on
  one engine.
* Chunk-0 blocks t<640 run in fp8 (e4m3) with DoubleRow perf mode
  (256-deep contraction, half the matmul passes and half the X bytes);
  late blocks and everything else stay bf16.  Decay weighting keeps
  the fp8 quantization error ~1.5% of max|Y| (gate 2e-2).
* Small chunks g>=4 also run in fp8 (their h decay fast; g1..g3 carry
  too much output mass and stay bf16): one fused DoubleRow GEMM per
  chunk off a tiny fp8 copy of the block-15 tail, one fused scan and
  one fused h-extract for all twelve.
* Two HWDGE queues stream the inputs: the scalar queue prefetches the
  small fp8 operands + first chunk-0 pair (it clears its preamble
  ~2us before sync), while the sync queue streams, in priority order:
  X15, SCN, remaining fp8 X pairs, bf16 W (chunks 0..3), W_ho hi half,
  bf16 X blocks 10..13, W_ho lo half (its projections run in the final
  scan chain's shadow).  The PE chases arrivals; the 16 output
  projections interleave into DMA slack instead of trailing at the end.
"""

import math
import os

from contextlib import ExitStack

import numpy as np

S, B, I, H, O = 1024, 64, 512, 2048, 512
NCORES = 8
BC = B // NCORES  # 8 batch rows per core
TB = 64  # X block granularity
WG = 32  # truncation-window granularity
NBLK = S // TB  # 16
NCH = H // 128  # 16 h-chunks
NI = I // 128  # 4 i-chunks
USMALL_W = 256  # max window (cols) for chunks g>=1
# chunk-0 scan segments as (first_block, n_blocks); full coverage of 0..15.
# Blocks 12/13 arrive last, so they get 1-block segments to shorten the
# serial scan tail; 14/15 (resident early for the small chunks) scan last.
SEGS = [(0, 2), (2, 2), (4, 2), (6, 2), (8, 2), (10, 2), (12, 1), (13, 3)]
GFUSE = 4  # small chunks g >= GFUSE share one u tile / one fused scan
FW = 32  # fused-chunk window (these chunks all truncate to one half-block)
HDR = 3  # per-row header cols: BIG sep (-1), BIG sep (+1), seed (-1)

_CACHE = {}


def _fp8_nblk():
    n = int(os.environ.get("DIAG_FP8BLK", "10"))
    assert n % 2 == 0 and 0 <= n <= (S - USMALL_W) // TB, n
    return n


def _seg_meta():
    """Per-segment (start_col, width) in per-row cols, and block->seg map."""
    seg_w = [HDR + nb * TB for _, nb in SEGS]
    seg_c = np.concatenate([[0], np.cumsum(seg_w)]).astype(int)
    seg_of = {}
    for s, (fb, nb) in enumerate(SEGS):
        for i in range(nb):
            seg_of[fb + i] = (s, i)
    return seg_w, seg_c, seg_of  # widths, col starts (len nseg+1), block map


def _register_scan_ops():
    """Fold op: m[t] = |m[t-1] - in0[t]*in1[t]|, zero-initialized."""
    import concourse.dve_ops as dve_ops
    from concourse.dve_spec import C0, Spec, Src0, Src1, Zero, scan, lower, AluOp
    from concourse.dve_uop import DveOpSpec

    have = {op.name: op for op in dve_ops.OPS}
    if "ABSDIFF_SCALE_SCAN_ANT" in have:
        return have["ABSDIFF_SCALE_SCAN_ANT"]

    def _ref(in0, in1, s0, s1, imm2):
        x = in0.astype(np.float32) * in1.astype(np.float32)
        out = np.empty_like(x)
        m = np.zeros(x.shape[0], np.float32)
        for t in range(x.shape[1]):
            m = np.abs(m - x[:, t])
            out[:, t] = m
        return out

    spec = Spec(
        body=scan(AluOp.ABSOLUTE_DIFF, Src0 * Src1, init=Zero),
        reference=_ref,
    )
    row = max(dve_ops._SUB_OPCODE_FOR_NAME.values()) + 1
    assert row < 0x20
    shas = {}
    for ver in ("v3", "v4"):
        s = DveOpSpec(
            name="ABSDIFF_SCALE_SCAN_ANT", opcode=row, uops=lower(spec, ver=ver),
            rd1_en=True,
        )
        shas[ver] = s.sha(ver)
    op = dve_ops.DveOp("ABSDIFF_SCALE_SCAN_ANT", spec, subdim=False, uops_sha=shas)
    dve_ops._SUB_OPCODE_FOR_NAME["ABSDIFF_SCALE_SCAN_ANT"] = row
    dve_ops.OPS.append(op)
    dve_ops.CUSTOM_DVE_SPECS["ABSDIFF_SCALE_SCAN_ANT"] = spec
    return op


def _windows(hh):
    ln = float(os.environ.get("DIAG_LN", "7.0"))
    a = np.maximum(np.abs(hh.astype(np.float64)), 1e-30)
    perm = np.argsort(-a, kind="stable")
    ag = a[perm].reshape(NCH, 128)
    windows = []
    for g in range(NCH):
        amax = ag[g, 0]
        if S * math.log(amax) >= -ln:
            kg = S
        else:
            kg = int(math.ceil(ln / math.log(1.0 / amax)))
        kg = min(S, max(WG, ((kg + WG - 1) // WG) * WG))
        windows.append(kg)
    assert windows[0] == S, windows
    assert all(windows[g] >= windows[g + 1] for g in range(NCH - 1)), windows
    assert all(k <= USMALL_W for k in windows[1:]), (windows, "raise USMALL_W")
    return perm, ag, tuple(windows)


def _small_offs(windows, base):
    """Start col of each small chunk's scn piece (g>=1), after chunk-0 base."""
    widths = [0] + [windows[g] + 2 for g in range(1, NCH)]
    return base + np.cumsum(widths).astype(int)  # index by g-1 ... use [g-1]


def _make_plan(hh):
    perm, ag, windows = _windows(hh)
    seg_w, seg_c, _ = _seg_meta()
    u0w = int(seg_c[-1])  # per-row cols of chunk-0 stream (1045)
    offs = np.concatenate(
        [[u0w], u0w + np.cumsum([windows[g] + 2 for g in range(1, NCH)])]
    ).astype(int)
    total = int(offs[-1])
    scn = np.zeros((128, total), dtype=np.float64)
    a0 = ag[0]
    col = 0
    for (fb, nb) in SEGS:
        scn[:, col] = -1.0
        scn[:, col + 1] = 1.0
        scn[:, col + 2] = -1.0
        t = np.arange(fb * TB, (fb + nb) * TB)
        scn[:, col + HDR : col + HDR + nb * TB] = -(
            a0[:, None] ** (S - 1 - t)[None, :]
        )
        col += HDR + nb * TB
    assert col == u0w
    for g in range(1, NCH):
        kg = windows[g]
        off = int(offs[g - 1])
        tau = np.arange(kg)
        scn[:, off : off + kg] = -(ag[g][:, None] ** (kg - 1 - tau)[None, :])
        scn[:, off + kg] = -1.0
        scn[:, off + kg + 1] = 1.0
    return {"perm": perm, "windows": windows, "offs": offs, "SCN": scn}


def _build(windows):
    import concourse.mybir as mybir
    import concourse.tile as tile
    from concourse import bacc
    from concourse.bass import ds

    SCAN_OP = _register_scan_ops()
    f32 = mybir.dt.float32
    bf16 = mybir.dt.bfloat16
    f8 = mybir.dt.float8e4
    DR = mybir.MatmulPerfMode.DoubleRow
    NF8 = _fp8_nblk()

    seg_w, seg_c, seg_of = _seg_meta()
    u0w = int(seg_c[-1])
    offs = np.concatenate(
        [[u0w], u0w + np.cumsum([windows[g] + 2 for g in range(1, NCH)])]
    ).astype(int)
    total_scn = int(offs[-1])
    nseg = len(SEGS)

    nc = bacc.Bacc("TRN2", target_bir_lowering=False, debug=False, num_devices=NCORES)
    XB = nc.dram_tensor(
        "XB", [NBLK - NF8, 128, NI * TB * BC], bf16, kind="ExternalInput"
    ).ap()  # blocks NF8..15, line [ic, b, tau]
    if NF8:
        # blocks 0..NF8-1 packed in pairs; per block the line is
        # [j, k, b, tau] with i = (2j+k)*128+p.  Pairing keeps 4KB DMA lines.
        X8 = nc.dram_tensor(
            "X8", [NF8 // 2, 128, 8 * TB * BC], f8, kind="ExternalInput"
        ).ap()
        W8 = nc.dram_tensor("W8", [128, 4 * 128], f8, kind="ExternalInput").ap()
    # fp8 operands for the fused small chunks: block-15 tail steps of X and
    # the g4..g15 weight rows, both in DoubleRow [j, k, ...] layout
    NFU_B = NCH - GFUSE
    X8S = nc.dram_tensor("X8S", [128, 4 * BC * WG], f8, kind="ExternalInput").ap()
    W8S = nc.dram_tensor("W8S", [128, NFU_B * 4 * 128], f8, kind="ExternalInput").ap()
    # bf16 input weights only needed for chunks 0..3 now
    WIHT = nc.dram_tensor("WIHT", [128, 4 * NI * 128], bf16, kind="ExternalInput").ap()
    WHOT = nc.dram_tensor("WHOT", [128, NCH * O], bf16, kind="ExternalInput").ap()
    SCN = nc.dram_tensor("SCN", [128, total_scn], bf16, kind="ExternalInput").ap()
    BIAS = nc.dram_tensor("BIAS", [BC, O], f32, kind="ExternalInput").ap()
    Y = nc.dram_tensor("Y", [BC, O], f32, kind="ExternalOutput").ap()

    with tile.TileContext(nc) as tc:
        with ExitStack() as ctx:
            consts = ctx.enter_context(tc.tile_pool(name="consts", bufs=1))
            xpool = ctx.enter_context(tc.tile_pool(name="xt", bufs=1))
            ubig = ctx.enter_context(tc.tile_pool(name="ubig", bufs=1))
            usmall = ctx.enter_context(tc.tile_pool(name="usmall", bufs=4))
            ypool = ctx.enter_context(tc.tile_pool(name="yout", bufs=1))
            gpool = ctx.enter_context(tc.tile_pool(name="gpsum", bufs=7, space="PSUM"))
            fpool = ctx.enter_context(tc.tile_pool(name="fpsum", bufs=1, space="PSUM"))

            wiht_t = consts.tile([128, 4 * NI * 128], bf16, tag="wiht", name="wiht_t")
            w8s_t = consts.tile([128, (NCH - GFUSE) * 4 * 128], f8, tag="w8s", name="w8s_t")
            x8s_t = consts.tile([128, 4 * BC * WG], f8, tag="x8s", name="x8s_t")
            whot_t = consts.tile([128, NCH * O], bf16, tag="whot", name="whot_t")
            scn_t = consts.tile([128, total_scn], bf16, tag="scn", name="scn_t")
            bias_t = ypool.tile([BC, O], f32, tag="bias", name="bias_t")
            h_all = consts.tile([128, NCH * BC], bf16, tag="hall", name="h_all")
            if NF8:
                w8_t = consts.tile([128, 4 * 128], f8, tag="w8", name="w8_t")
            # fp8 X packed two blocks per tile (4KB DMA lines for full rate)
            x8pts = [
                xpool.tile([128, 8 * TB * BC], f8, tag=f"x8p_{pi}", name=f"x8p_{pi}")[:]
                for pi in range(NF8 // 2)
            ]

            def x8_half(kb):  # [p, (j,k,b,tau)] slice for one block
                return x8pts[kb // 2][:, ds((kb % 2) * 4 * TB * BC, 4 * TB * BC)]

            xbts = {
                kb: xpool.tile(
                    [128, NI * TB * BC], bf16, tag=f"xb_{kb}", name=f"xb_{kb}"
                )[:]
                for kb in range(NF8, NBLK)
            }
            u0_t = ubig.tile([128, BC * u0w], f32, tag="u0", name="u0")
            # fused u tile for small chunks g in [GFUSE, 16): [p, (b, c, FW+2)].
            # b-major so the fused scan is rank-3: [p, b, (c t)] with the scn
            # stream broadcast over b; each (b, c) sub-stream is independent.
            NFU = NCH - GFUSE
            u12_t = ubig.tile(
                [128, BC * NFU * (FW + 2)], f32, tag="u12", name="u12"
            )

            def u12_4d():  # [p, b, c, t]
                return u12_t[:].rearrange(
                    "p (b c t) -> p b c t", b=BC, c=NFU
                )

            def u0_seg(s):  # [p, b, width_s]
                return u0_t[:, ds(int(seg_c[s]) * BC, BC * seg_w[s])].rearrange(
                    "p (b t) -> p b t", b=BC
                )

            # --- DMA stream (single HWDGE queue; order = priority).
            # fp8 chunk-0 blocks go first so the GEMM->scan chain chases the
            # stream from the start; small-chunk weights and X14/15 interleave
            # to fill PE slack; chunk-0-only bf16 blocks 10..13 stream last. ---
            def wp(g0, ng):
                return ds(g0 * NI * 128, ng * NI * 128)

            def xbd(kb):
                nc.sync.dma_start(xbts[kb], XB[kb - NF8])

            dma = nc.sync.dma_start
            assert NF8 == 10, "stream schedule is tuned for DIAG_FP8BLK=10"
            # Prefetch the small-chunk fp8 operands + first chunk-0 pair on
            # the scalar HWDGE queue: it clears its preamble ~2us before sync
            # and doubles early DMA concurrency through the ramp.  A single
            # in-flight DMA tops out well below the aggregate rate, so W8S is
            # split in two.
            nc.scalar.dma_start(w8s_t[:], W8S)
            nc.scalar.dma_start(x8s_t[:], X8S)
            nc.scalar.dma_start(w8_t[:], W8)
            nc.scalar.dma_start(x8pts[0], X8[0])
            xbd(15)
            dma(scn_t[:], SCN)
            dma(x8pts[1], X8[1])
            xbd(14)
            dma(x8pts[2], X8[2])
            dma(x8pts[3], X8[3])
            dma(wiht_t[:], WIHT)  # bf16 weights for chunks 0..3
            dma(x8pts[4], X8[4])
            dma(whot_t[:, ds(8 * O, 8 * O)], WHOT[:, ds(8 * O, 8 * O)])  # g8..15
            xbd(10)
            xbd(11)
            xbd(12)
            xbd(13)
            # lo half of W_ho streams last: its projections run in the shadow
            # of the final scan chain, while XB12/13 (which gate that chain)
            # arrive earlier
            dma(whot_t[:, ds(0, 8 * O)], WHOT[:, ds(0, 8 * O)])  # g0..7
            dma(bias_t[:], BIAS)

            # --- header memsets for the fused scans ---
            for s in range(nseg):
                nc.gpsimd.memset(u0_seg(s)[:, :, ds(0, 2)], 1.0e30)
            nc.gpsimd.memset(u0_seg(0)[:, :, ds(2, 1)], 0.0)

            # --- PE warm-up: the HAM clock gate lifts a fixed ~5us after the
            # first matmul, so start that timer as early and cheaply as
            # possible (bf16: 1 cycle/col; fp32 warms cost 4x and queue
            # ahead of real work on the in-order PE) ---
            warm = consts.tile([128, TB * BC], bf16, tag="warm", name="warm")
            nc.gpsimd.memset(warm[:], 0.0)
            wps = gpool.tile([128, TB * BC], f32, tag="gp", name="warm_ps")
            NWARM = 2
            for i in range(NWARM):
                nc.tensor.matmul(
                    wps[:], warm[:, ds(0, 128)], warm[:],
                    start=(i == 0), stop=(i == NWARM - 1),
                )
            nc.scalar.copy(warm[:], wps[:])

            # --- output projection bookkeeping ---
            psy = fpool.tile([BC, O], f32, tag="fy", name="psy")
            proj_pending = list(range(NCH - 1, 0, -1))  # g15..g1; g0 last
            proj_done = [0]

            def emit_proj_g(g):
                nc.tensor.matmul(
                    psy[:], h_all[:, ds(g * BC, BC)], whot_t[:, ds(g * O, O)],
                    start=(proj_done[0] == 0), stop=(proj_done[0] == NCH - 1),
                )
                proj_done[0] += 1

            def emit_projs(n):
                for _ in range(min(n, len(proj_pending))):
                    emit_proj_g(proj_pending.pop(0))

            # --- small chunks: one GEMM unit per g.  g >= GFUSE (window WG,
            # block 15 only) write into the shared u12 tile and are scanned /
            # extracted by ONE fused instruction each at the end. ---
            for g in range(GFUSE, NCH):
                assert windows[g] == FW, (g, windows)
            nc.gpsimd.memset(
                u12_t[:].rearrange("p (x t) -> p x t", t=FW + 2)[:, :, ds(FW, 2)],
                1.0e30,
            )

            def emit_small_gemm(g):
                kg = windows[g]
                st0 = S - kg
                fb = st0 // TB
                toff = st0 % TB
                if g >= GFUSE:
                    u3 = u12_4d()[:, :, g - GFUSE]  # [p, b, WG+2]
                else:
                    u_t = usmall.tile(
                        [128, BC * (USMALL_W + 2)], f32, tag="us", name=f"u_g{g}"
                    )
                    u3 = u_t[:, ds(0, BC * (kg + 2))].rearrange(
                        "p (b t) -> p b t", b=BC
                    )
                    nc.gpsimd.memset(u3[:, :, ds(kg, 2)], 1.0e30)
                blocks = list(range(fb, NBLK))

                def t0_of(kb, fb=fb, toff=toff):
                    return toff if kb == fb else 0

                ps = {
                    kb: gpool.tile(
                        [128, (TB - t0_of(kb)) * BC], f32, tag="gp", name=f"gp_{g}_{kb}"
                    )
                    for kb in blocks
                }
                for ic in range(NI):
                    for kb in blocks:
                        t0 = t0_of(kb)
                        rhs = xbts[kb][:, ds(ic * TB * BC, TB * BC)]
                        out_ap = ps[kb][:]
                        if t0:
                            rhs = rhs.rearrange("p (b t) -> p b t", b=BC)[
                                :, :, ds(t0, TB - t0)
                            ]
                            out_ap = out_ap.rearrange("p (b t) -> p b t", b=BC)
                        nc.tensor.matmul(
                            out_ap,
                            wiht_t[:, ds(g * NI * 128 + ic * 128, 128)],
                            rhs,
                            start=(ic == 0),
                            stop=(ic == NI - 1),
                        )
                for kb in blocks:
                    t0 = t0_of(kb)
                    pos = kb * TB - st0 if kb > fb else 0
                    nc.scalar.copy(
                        u3[:, :, ds(pos, TB - t0)],
                        ps[kb][:].rearrange("p (b t) -> p b t", b=BC),
                    )
                return u3

            def emit_small(g):  # unfused path: GEMM + its own scan + extract
                kg = windows[g]
                u3 = emit_small_gemm(g)
                scn_g = (
                    scn_t[:, ds(int(offs[g - 1]), kg + 2)]
                    .rearrange("p (o t) -> p o t", o=1)
                    .broadcast_to([128, BC, kg + 2])
                )
                nc.vector._custom_dve(SCAN_OP, out=u3, in0=u3, in1=scn_g)
                nc.vector.tensor_copy(h_all[:, ds(g * BC, BC)], u3[:, :, kg - 1])

            def emit_small_f8(g):
                # fp8 DoubleRow GEMM for one fused chunk: X = block-15 tail
                # steps (x8s), weights from w8s; two j-passes of k=256
                c = g - GFUSE
                ps = gpool.tile([128, WG * BC], f32, tag="gp", name=f"gp8_{g}")
                for j in range(2):
                    w_ap = w8s_t[
                        :, ds(c * 4 * 128 + j * 2 * 128, 2 * 128)
                    ].rearrange("p (k h) -> p k h", k=2)
                    rhs = x8s_t[:, ds(j * 2 * BC * WG, 2 * BC * WG)].rearrange(
                        "p (k n) -> p k n", k=2
                    )
                    nc.tensor.matmul(
                        ps[:], w_ap, rhs,
                        start=(j == 0), stop=(j == 1), perf_mode=DR,
                    )
                nc.scalar.copy(
                    u12_4d()[:, :, c, ds(0, WG)],
                    ps[:].rearrange("p (b t) -> p b t", b=BC),
                )

            def emit_fused_small_scan():
                u3 = u12_t[:].rearrange(
                    "p (b ct) -> p b ct", b=BC
                )  # [p, b, NFU*(FW+2)]
                scn12 = (
                    scn_t[:, ds(int(offs[GFUSE - 1]), NFU * (FW + 2))]
                    .rearrange("p (o t) -> p o t", o=1)
                    .broadcast_to([128, BC, NFU * (FW + 2)])
                )
                nc.vector._custom_dve(SCAN_OP, out=u3, in0=u3, in1=scn12)
                # h extract for all fused chunks at once: dst is (c, b)-major,
                # so read the (b, c, t)-tile with c outer via strides
                src = u12_t[:].rearrange(
                    "p (b c t) -> p c b t", b=BC, c=NFU
                )[:, :, :, FW - 1]
                nc.vector.tensor_copy(h_all[:, ds(GFUSE * BC, NFU * BC)], src)

            # --- chunk 0: GEMM chases the DMA stream; fused per-seg scans ---
            seg_left = {s: nb for s, (_, nb) in enumerate(SEGS)}
            next_scan = [0]

            def copy_c0(kb, ps_t):
                s, i = seg_of[kb]
                nc.scalar.copy(
                    u0_seg(s)[:, :, ds(HDR + i * TB, TB)],
                    ps_t[:].rearrange("p (b t) -> p b t", b=BC),
                )
                seg_left[s] -= 1

            def emit_ready_scans():
                while next_scan[0] < nseg and seg_left[next_scan[0]] == 0:
                    s = next_scan[0]
                    w = seg_w[s]
                    if s > 0:
                        nc.vector.tensor_copy(
                            u0_seg(s)[:, :, 2], u0_seg(s - 1)[:, :, seg_w[s - 1] - 1]
                        )
                    scn_s = (
                        scn_t[:, ds(int(seg_c[s]), w)]
                        .rearrange("p (o t) -> p o t", o=1)
                        .broadcast_to([128, BC, w])
                    )
                    u3s = u0_seg(s)
                    nc.vector._custom_dve(SCAN_OP, out=u3s, in0=u3s, in1=scn_s)
                    next_scan[0] += 1

            def emit_c0_bf16_run(blocks):
                pss = {
                    kb: gpool.tile([128, TB * BC], f32, tag="gp", name=f"c0_{kb}")
                    for kb in blocks
                }
                for ic in range(NI):
                    for kb in blocks:
                        nc.tensor.matmul(
                            pss[kb][:],
                            wiht_t[:, ds(ic * 128, 128)],
                            xbts[kb][:, ds(ic * TB * BC, TB * BC)],
                            start=(ic == 0),
                            stop=(ic == NI - 1),
                        )
                for kb in blocks:
                    copy_c0(kb, pss[kb])

            def emit_c0_fp8_pair(k0):
                pair = (k0, k0 + 1)
                pss = {
                    kb: gpool.tile([128, TB * BC], f32, tag="gp", name=f"c8_{kb}")
                    for kb in pair
                }
                for j in range(2):
                    w_ap = w8_t[:].rearrange("p (j k h) -> p j k h", j=2, k=2)[:, j]
                    for kb in pair:
                        rhs = x8_half(kb).rearrange("p (j k n) -> p j k n", j=2, k=2)[
                            :, j
                        ]
                        nc.tensor.matmul(
                            pss[kb][:], w_ap, rhs,
                            start=(j == 0), stop=(j == 1), perf_mode=DR,
                        )
                for kb in pair:
                    copy_c0(kb, pss[kb])

            # --- interleaved schedule: the tiny fp8 small-chunk operands
            # arrive first, so all 12 fused small GEMMs run during the fp8
            # chunk-0 stream ramp; chunk-0 pairs chase the stream; bf16
            # blocks 10..13 trail; projections ride in DMA/scan shadows. ---
            for g in range(NCH - 1, GFUSE - 1, -1):
                emit_small_f8(g)
            emit_fused_small_scan()  # one scan + one extract for g4..g15
            emit_c0_fp8_pair(0)
            emit_ready_scans()
            emit_c0_fp8_pair(2)
            emit_ready_scans()
            emit_c0_fp8_pair(4)
            emit_ready_scans()
            emit_small(3)
            emit_small(2)
            emit_c0_fp8_pair(6)
            emit_ready_scans()
            emit_small(1)
            emit_c0_fp8_pair(8)
            emit_ready_scans()
            emit_c0_bf16_run([14, 15])  # resident early; tail-seg prefill
            emit_projs(4)  # g15..g12 (WHOT hi half)
            emit_c0_bf16_run([10])
            emit_ready_scans()
            emit_projs(2)
            emit_c0_bf16_run([11])
            emit_ready_scans()
            emit_projs(2)
            emit_c0_bf16_run([12])
            emit_ready_scans()
            emit_c0_bf16_run([13])
            emit_ready_scans()
            # remaining g-projections run in the shadow of the final segment
            # scans, fed by the late-arriving lo half of W_ho
            emit_projs(len(proj_pending))
            assert next_scan[0] == nseg and not any(seg_left.values()), (
                next_scan, seg_left,
            )
            # chunk-0 h extract, then its projection closes the accumulation
            nc.vector.tensor_copy(
                h_all[:, ds(0, BC)], u0_seg(nseg - 1)[:, :, seg_w[nseg - 1] - 1]
            )
            emit_proj_g(0)
            assert proj_done[0] == NCH

            y_t = ypool.tile([BC, O], f32, tag="y", name="y_t")
            nc.vector.tensor_tensor(y_t[:], psy[:], bias_t[:], mybir.AluOpType.add)
            nc.sync.dma_start(Y, y_t[:])
    nc.compile()
    return nc


def _get_program(windows):
    key = (windows, os.environ.get("DIAG_LN"), os.environ.get("DIAG_FP8BLK"))
    if key not in _CACHE:
        _CACHE[key] = _build(windows)
    return _CACHE[key]


def _ensure_ntff_hook():
    """Provide antenv.axon_hooks (absent in this image) so trace=True works."""
    import sys
    import types

    if "antenv.axon_hooks" in sys.modules:
        return True
    try:
        import antenv

        mod = types.ModuleType("antenv.axon_hooks")
        mod._hook = None

        def set_axon_ntff_profile_hook(h):
            mod._hook = h

        def get_axon_ntff_profile_hook():
            return mod._hook

        mod.set_axon_ntff_profile_hook = set_axon_ntff_profile_hook
        mod.get_axon_ntff_profile_hook = get_axon_ntff_profile_hook
        sys.modules["antenv.axon_hooks"] = mod
        antenv.axon_hooks = mod

        from trn_agent_boot.trn_boot import _ntff_profile_via_ctypes

        hook = _ntff_profile_via_ctypes("/opt/axon/libaxon_pjrt.so")
        mod.set_axon_ntff_profile_hook(hook)
        return hook is not None
    except Exception:
        return False


def kernel(X, W_ih, hh, W_ho, b_ho):
    import ml_dtypes

    from concourse import bass_utils

    X = np.asarray(X, dtype=np.float32)
    W_ih = np.asarray(W_ih, dtype=np.float32)
    hh = np.asarray(hh, dtype=np.float32)
    W_ho = np.asarray(W_ho, dtype=np.float32)
    b_ho = np.asarray(b_ho, dtype=np.float32)

    plan = _make_plan(hh)
    perm = plan["perm"]
    nc = _get_program(plan["windows"])
    NF8 = _fp8_nblk()

    bf = ml_dtypes.bfloat16
    f8 = ml_dtypes.float8_e4m3
    # WIHT [128, 4*NI*128]: line p = [g, ic, hsub] for chunks 0..3 only
    wiht = np.ascontiguousarray(
        W_ih[perm[: 4 * 128]]
        .T.reshape(NI, 128, 4, 128)
        .transpose(1, 2, 0, 3)
        .reshape(128, -1)
    ).astype(bf)
    # W8S [128, (c, j, k, h)]: fp8 weights for fused chunks g in [GFUSE, 16)
    w8s = np.ascontiguousarray(
        W_ih[perm[GFUSE * 128 :]]
        .reshape(NCH - GFUSE, 128, 2, 2, 128)  # [c, h, j, k, p]
        .transpose(4, 0, 2, 3, 1)
        .reshape(128, -1)
    ).astype(f8)
    # WHOT [128, NCH*O]: line p = [g, o] with value W_ho[o, h=g*128+p]
    whot = np.ascontiguousarray(
        W_ho[:, perm].T.reshape(NCH, 128, O).transpose(1, 0, 2).reshape(128, NCH * O)
    ).astype(bf)
    bias = np.tile(b_ho[None, :], (BC, 1)).astype(np.float32)

    common = {
        "WIHT": wiht,
        "W8S": w8s,
        "WHOT": whot,
        "BIAS": bias,
        "SCN": plan["SCN"].astype(bf),
    }
    if NF8:
        # W8 [128, (j,k,h)]: chunk-0 weights, i = (2j+k)*128 + p
        w0 = W_ih[perm[:128]]  # [128h, I]
        common["W8"] = np.ascontiguousarray(
            w0.T.reshape(2, 2, 128, 128).transpose(2, 0, 1, 3).reshape(128, 512)
        ).astype(f8)
    in_maps = []
    for m in range(NCORES):
        im = dict(common)
        xm = X[:, m * BC : (m + 1) * BC, :]  # [S, BC, I]
        # bf16 blocks: [128(i%128), (ic, b, tau)]
        xt = xm.transpose(2, 1, 0).reshape(NI, 128, BC, NBLK, TB)
        xt = np.ascontiguousarray(xt.transpose(3, 1, 0, 2, 4)).reshape(
            NBLK, 128, NI * BC * TB
        )
        im["XB"] = xt[NF8:].astype(bf)
        # X8S [128, (j, k, b, tau)]: fp8 copy of the last WG steps (block 15
        # tail) feeding the fused small-chunk GEMMs
        im["X8S"] = np.ascontiguousarray(
            xm[S - WG :]
            .transpose(2, 1, 0)  # [i, b, tau]
            .reshape(2, 2, 128, BC, WG)  # [j, k, p, b, tau]
            .transpose(2, 0, 1, 3, 4)
            .reshape(128, 4 * BC * WG)
        ).astype(f8)
        if NF8:
            x8 = np.empty((NF8, 128, 4 * BC * TB), dtype=f8)
            for kb in range(NF8):
                blk = xm[kb * TB : (kb + 1) * TB]  # [TB, BC, I]
                a = blk.transpose(2, 1, 0).reshape(2, 2, 128, BC, TB)  # [j,k,p,b,t]
                x8[kb] = (
                    np.ascontiguousarray(a.transpose(2, 0, 1, 3, 4))
                    .reshape(128, 4 * BC * TB)
                    .astype(f8)
                )
            # pack block pairs along the line dim: [NF8//2, 128, 8*BC*TB]
            im["X8"] = np.ascontiguousarray(
                x8.reshape(NF8 // 2, 2, 128, 4 * BC * TB).transpose(0, 2, 1, 3)
            ).reshape(NF8 // 2, 128, 8 * BC * TB)
        in_maps.append(im)

    trace = bool(int(os.environ.get("DIAG_TRACE", "0")))
    if trace:
        trace = _ensure_ntff_hook()
    res = None
    for attempt in range(3):
        try:
            res = bass_utils.run_bass_kernel_spmd(
                nc,
                in_maps,
                core_ids=list(range(NCORES)),
                trace=trace,
                tmpdir=os.environ.get("DIAG_TRACE_DIR") or None,
            )
            break
        except Exception:
            if attempt == 2:
                raise
            trace = False  # retry without profiling
    if res.exec_time_ns is not None:
        kernel.last_exec_time_ns = res.exec_time_ns
        kernel.last_mean_exec_time_ns = res.mean_exec_time_ns
    Yfull = np.concatenate([r["Y"] for r in res.results], axis=0)
    return Yfull


kernel.last_exec_time_ns = None
kernel.last_mean_exec_time_ns = None


# revision 62
# speedup vs baseline: 1.0311x; 1.0311x over previous
"""Trainium2 Bass kernel for nn_Diagnet (S=1024, B=64, I=512, H=2048, O=512).

    u = einsum('sbi,hi->sbh', X, W_ih)
    h_t = |u_t + hh * h_{t-1}|   (scan over S, only final h needed)
    Y = h_final @ W_ho.T + b_ho

Strategy (8 NeuronCores, data-parallel over batch, BC=8 rows per core):

* H lanes are permuted so hh is sorted descending and split into 16
  chunks of 128.  A chunk whose largest decay a satisfies a^K < tol
  only needs the last K steps, so each chunk gets a window K_g and the
  GEMM + scan skip everything earlier.  Chunk 0 keeps the full 1024.
* The recurrence is a custom DVE instruction that folds a window in
  one go: m[t] = |m[t-1] - u[t]*scn[t]| with scn[t] = -a^(K-1-t)
  (prescale folds the decay into the stream; the minus sign turns
  ABSOLUTE_DIFF into abs-add).  h_final = last element.
* All 8 batch rows fold in ONE scan instruction per segment via a
  3-column header per row: a BIG separator pair scaled (-1, +1)
  absorbs and exactly zeroes the running state between rows, then a
  seed column (scn=-1) re-injects that row's carry from the previous
  segment (h >= 0 so |0 - s*(-1)| = s).  Seed values are copied
  between segments by a tiny DVE copy, keeping the serial chain on
  one engine.
* Chunk-0 blocks t<640 run in fp8 (e4m3) with DoubleRow perf mode
  (256-deep contraction, half the matmul passes and half the X bytes);
  late blocks and everything else stay bf16.  Decay weighting keeps
  the fp8 quantization error ~1.5% of max|Y| (gate 2e-2).
* Small chunks g>=4 also run in fp8 (their h decay fast; g1..g3 carry
  too much output mass and stay bf16): one fused DoubleRow GEMM per
  chunk off a tiny fp8 copy of the block-15 tail, one fused scan and
  one fused h-extract for all twelve.
* Two HWDGE queues stream the inputs: the scalar queue prefetches the
  small fp8 operands + first chunk-0 pair (it clears its preamble
  ~2us before sync), while the sync queue streams, in priority order:
  X15, SCN, remaining fp8 X pairs, bf16 W (chunks 0..3), W_ho hi half,
  bf16 X blocks 10..13, W_ho lo half (its projections run in the final
  scan chain's shadow).  The PE chases arrivals; the 16 output
  projections interleave into DMA slack instead of trailing at the end.
"""

import math
import os

from contextlib import ExitStack

import numpy as np

S, B, I, H, O = 1024, 64, 512, 2048, 512
NCORES = 8
BC = B // NCORES  # 8 batch rows per core
TB = 64  # X block granularity
WG = 32  # truncation-window granularity
NBLK = S // TB  # 16
NCH = H // 128  # 16 h-chunks
NI = I // 128  # 4 i-chunks
USMALL_W = 256  # max window (cols) for chunks g>=1
# chunk-0 scan segments as (first_block, n_blocks); full coverage of 0..15.
# Blocks 12/13 arrive last, so they get 1-block segments to shorten the
# serial scan tail; 14/15 (resident early for the small chunks) scan last.
SEGS = [(0, 2), (2, 2), (4, 2), (6, 2), (8, 2), (10, 2), (12, 1), (13, 3)]
GFUSE = 4  # small chunks g >= GFUSE share one u tile / one fused scan
FW = 32  # fused-chunk window (these chunks all truncate to one half-block)
HDR = 3  # per-row header cols: BIG sep (-1), BIG sep (+1), seed (-1)

_CACHE = {}


def _fp8_nblk():
    n = int(os.environ.get("DIAG_FP8BLK", "10"))
    assert n % 2 == 0 and 0 <= n <= (S - USMALL_W) // TB, n
    return n


def _seg_meta():
    """Per-segment (start_col, width) in per-row cols, and block->seg map."""
    seg_w = [HDR + nb * TB for _, nb in SEGS]
    seg_c = np.concatenate([[0], np.cumsum(seg_w)]).astype(int)
    seg_of = {}
    for s, (fb, nb) in enumerate(SEGS):
        for i in range(nb):
            seg_of[fb + i] = (s, i)
    return seg_w, seg_c, seg_of  # widths, col starts (len nseg+1), block map


def _register_scan_ops():
    """Fold op: m[t] = |m[t-1] - in0[t]*in1[t]|, zero-initialized."""
    import concourse.dve_ops as dve_ops
    from concourse.dve_spec import C0, Spec, Src0, Src1, Zero, scan, lower, AluOp
    from concourse.dve_uop import DveOpSpec

    have = {op.name: op for op in dve_ops.OPS}
    if "ABSDIFF_SCALE_SCAN_ANT" in have:
        return have["ABSDIFF_SCALE_SCAN_ANT"]

    def _ref(in0, in1, s0, s1, imm2):
        x = in0.astype(np.float32) * in1.astype(np.float32)
        out = np.empty_like(x)
        m = np.zeros(x.shape[0], np.float32)
        for t in range(x.shape[1]):
            m = np.abs(m - x[:, t])
            out[:, t] = m
        return out

    spec = Spec(
        body=scan(AluOp.ABSOLUTE_DIFF, Src0 * Src1, init=Zero),
        reference=_ref,
    )
    row = max(dve_ops._SUB_OPCODE_FOR_NAME.values()) + 1
    assert row < 0x20
    shas = {}
    for ver in ("v3", "v4"):
        s = DveOpSpec(
            name="ABSDIFF_SCALE_SCAN_ANT", opcode=row, uops=lower(spec, ver=ver),
            rd1_en=True,
        )
        shas[ver] = s.sha(ver)
    op = dve_ops.DveOp("ABSDIFF_SCALE_SCAN_ANT", spec, subdim=False, uops_sha=shas)
    dve_ops._SUB_OPCODE_FOR_NAME["ABSDIFF_SCALE_SCAN_ANT"] = row
    dve_ops.OPS.append(op)
    dve_ops.CUSTOM_DVE_SPECS["ABSDIFF_SCALE_SCAN_ANT"] = spec
    return op


def _windows(hh):
    ln = float(os.environ.get("DIAG_LN", "7.0"))
    a = np.maximum(np.abs(hh.astype(np.float64)), 1e-30)
    perm = np.argsort(-a, kind="stable")
    ag = a[perm].reshape(NCH, 128)
    windows = []
    for g in range(NCH):
        amax = ag[g, 0]
        if S * math.log(amax) >= -ln:
            kg = S
        else:
            kg = int(math.ceil(ln / math.log(1.0 / amax)))
        kg = min(S, max(WG, ((kg + WG - 1) // WG) * WG))
        windows.append(kg)
    assert windows[0] == S, windows
    assert all(windows[g] >= windows[g + 1] for g in range(NCH - 1)), windows
    assert all(k <= USMALL_W for k in windows[1:]), (windows, "raise USMALL_W")
    return perm, ag, tuple(windows)


def _small_offs(windows, base):
    """Start col of each small chunk's scn piece (g>=1), after chunk-0 base."""
    widths = [0] + [windows[g] + 2 for g in range(1, NCH)]
    return base + np.cumsum(widths).astype(int)  # index by g-1 ... use [g-1]


def _make_plan(hh):
    perm, ag, windows = _windows(hh)
    seg_w, seg_c, _ = _seg_meta()
    u0w = int(seg_c[-1])  # per-row cols of chunk-0 stream (1045)
    offs = np.concatenate(
        [[u0w], u0w + np.cumsum([windows[g] + 2 for g in range(1, NCH)])]
    ).astype(int)
    total = int(offs[-1])
    scn = np.zeros((128, total), dtype=np.float64)
    a0 = ag[0]
    col = 0
    for (fb, nb) in SEGS:
        scn[:, col] = -1.0
        scn[:, col + 1] = 1.0
        scn[:, col + 2] = -1.0
        t = np.arange(fb * TB, (fb + nb) * TB)
        scn[:, col + HDR : col + HDR + nb * TB] = -(
            a0[:, None] ** (S - 1 - t)[None, :]
        )
        col += HDR + nb * TB
    assert col == u0w
    for g in range(1, NCH):
        kg = windows[g]
        off = int(offs[g - 1])
        tau = np.arange(kg)
        scn[:, off : off + kg] = -(ag[g][:, None] ** (kg - 1 - tau)[None, :])
        scn[:, off + kg] = -1.0
        scn[:, off + kg + 1] = 1.0
    return {"perm": perm, "windows": windows, "offs": offs, "SCN": scn}


def _build(windows):
    import concourse.mybir as mybir
    import concourse.tile as tile
    from concourse import bacc
    from concourse.bass import ds

    SCAN_OP = _register_scan_ops()
    f32 = mybir.dt.float32
    bf16 = mybir.dt.bfloat16
    f8 = mybir.dt.float8e4
    DR = mybir.MatmulPerfMode.DoubleRow
    NF8 = _fp8_nblk()

    seg_w, seg_c, seg_of = _seg_meta()
    u0w = int(seg_c[-1])
    offs = np.concatenate(
        [[u0w], u0w + np.cumsum([windows[g] + 2 for g in range(1, NCH)])]
    ).astype(int)
    total_scn = int(offs[-1])
    nseg = len(SEGS)

    nc = bacc.Bacc("TRN2", target_bir_lowering=False, debug=False, num_devices=NCORES)
    XB = nc.dram_tensor(
        "XB", [NBLK - NF8, 128, NI * TB * BC], bf16, kind="ExternalInput"
    ).ap()  # blocks NF8..15, line [ic, b, tau]
    if NF8:
        # blocks 0..NF8-1 packed in pairs; per block the line is
        # [j, k, b, tau] with i = (2j+k)*128+p.  Pairing keeps 4KB DMA lines.
        X8 = nc.dram_tensor(
            "X8", [NF8 // 2, 128, 8 * TB * BC], f8, kind="ExternalInput"
        ).ap()
        W8 = nc.dram_tensor("W8", [128, 4 * 128], f8, kind="ExternalInput").ap()
    # fp8 operands for the fused small chunks: block-15 tail steps of X and
    # the g4..g15 weight rows, both in DoubleRow [j, k, ...] layout
    NFU_B = NCH - GFUSE
    X8S = nc.dram_tensor("X8S", [128, 4 * BC * WG], f8, kind="ExternalInput").ap()
    W8S = nc.dram_tensor("W8S", [128, NFU_B * 4 * 128], f8, kind="ExternalInput").ap()
    # bf16 input weights only needed for chunks 0..3 now
    WIHT = nc.dram_tensor("WIHT", [128, 4 * NI * 128], bf16, kind="ExternalInput").ap()
    WHOT = nc.dram_tensor("WHOT", [128, NCH * O], bf16, kind="ExternalInput").ap()
    SCN = nc.dram_tensor("SCN", [128, total_scn], bf16, kind="ExternalInput").ap()
    BIAS = nc.dram_tensor("BIAS", [BC, O], f32, kind="ExternalInput").ap()
    Y = nc.dram_tensor("Y", [BC, O], f32, kind="ExternalOutput").ap()

    with tile.TileContext(nc) as tc:
        with ExitStack() as ctx:
            consts = ctx.enter_context(tc.tile_pool(name="consts", bufs=1))
            xpool = ctx.enter_context(tc.tile_pool(name="xt", bufs=1))
            ubig = ctx.enter_context(tc.tile_pool(name="ubig", bufs=1))
            usmall = ctx.enter_context(tc.tile_pool(name="usmall", bufs=4))
            ypool = ctx.enter_context(tc.tile_pool(name="yout", bufs=1))
            gpool = ctx.enter_context(tc.tile_pool(name="gpsum", bufs=7, space="PSUM"))
            fpool = ctx.enter_context(tc.tile_pool(name="fpsum", bufs=1, space="PSUM"))

            wiht_t = consts.tile([128, 4 * NI * 128], bf16, tag="wiht", name="wiht_t")
            w8s_t = consts.tile([128, (NCH - GFUSE) * 4 * 128], f8, tag="w8s", name="w8s_t")
            x8s_t = consts.tile([128, 4 * BC * WG], f8, tag="x8s", name="x8s_t")
            whot_t = consts.tile([128, NCH * O], bf16, tag="whot", name="whot_t")
            scn_t = consts.tile([128, total_scn], bf16, tag="scn", name="scn_t")
            bias_t = ypool.tile([BC, O], f32, tag="bias", name="bias_t")
            h_all = consts.tile([128, NCH * BC], bf16, tag="hall", name="h_all")
            if NF8:
                w8_t = consts.tile([128, 4 * 128], f8, tag="w8", name="w8_t")
            # fp8 X packed two blocks per tile (4KB DMA lines for full rate)
            x8pts = [
                xpool.tile([128, 8 * TB * BC], f8, tag=f"x8p_{pi}", name=f"x8p_{pi}")[:]
                for pi in range(NF8 // 2)
            ]

            def x8_half(kb):  # [p, (j,k,b,tau)] slice for one block
                return x8pts[kb // 2][:, ds((kb % 2) * 4 * TB * BC, 4 * TB * BC)]

            xbts = {
                kb: xpool.tile(
                    [128, NI * TB * BC], bf16, tag=f"xb_{kb}", name=f"xb_{kb}"
                )[:]
                for kb in range(NF8, NBLK)
            }
            u0_t = ubig.tile([128, BC * u0w], f32, tag="u0", name="u0")
            # fused u tile for small chunks g in [GFUSE, 16): [p, (b, c, FW+2)].
            # b-major so the fused scan is rank-3: [p, b, (c t)] with the scn
            # stream broadcast over b; each (b, c) sub-stream is independent.
            NFU = NCH - GFUSE
            u12_t = ubig.tile(
                [128, BC * NFU * (FW + 2)], f32, tag="u12", name="u12"
            )

            def u12_4d():  # [p, b, c, t]
                return u12_t[:].rearrange(
                    "p (b c t) -> p b c t", b=BC, c=NFU
                )

            def u0_seg(s):  # [p, b, width_s]
                return u0_t[:, ds(int(seg_c[s]) * BC, BC * seg_w[s])].rearrange(
                    "p (b t) -> p b t", b=BC
                )

            # --- DMA stream (single HWDGE queue; order = priority).
            # fp8 chunk-0 blocks go first so the GEMM->scan chain chases the
            # stream from the start; small-chunk weights and X14/15 interleave
            # to fill PE slack; chunk-0-only bf16 blocks 10..13 stream last. ---
            def wp(g0, ng):
                return ds(g0 * NI * 128, ng * NI * 128)

            def xbd(kb):
                nc.sync.dma_start(xbts[kb], XB[kb - NF8])

            dma = nc.sync.dma_start
            assert NF8 == 10, "stream schedule is tuned for DIAG_FP8BLK=10"
            # Prefetch the small-chunk fp8 operands + first chunk-0 pair on
            # the scalar HWDGE queue: it clears its preamble ~2us before sync
            # and doubles early DMA concurrency through the ramp.  A single
            # in-flight DMA tops out well below the aggregate rate, so W8S is
            # split in two.
            nc.scalar.dma_start(w8s_t[:], W8S)
            nc.scalar.dma_start(x8s_t[:], X8S)
            nc.scalar.dma_start(w8_t[:], W8)
            nc.scalar.dma_start(x8pts[0], X8[0])
            xbd(15)
            dma(scn_t[:], SCN)
            dma(x8pts[1], X8[1])
            xbd(14)
            dma(x8pts[2], X8[2])
            dma(x8pts[3], X8[3])
            dma(wiht_t[:], WIHT)  # bf16 weights for chunks 0..3
            dma(x8pts[4], X8[4])
            dma(whot_t[:, ds(8 * O, 8 * O)], WHOT[:, ds(8 * O, 8 * O)])  # g8..15
            xbd(10)
            xbd(11)
            # blocks 12/13 gate the final scan chain: stream them split by
            # i-chunk so their ic-accumulating GEMMs pipeline with arrival
            for kb in (12, 13):
                for ic in range(NI):
                    nc.sync.dma_start(
                        xbts[kb][:, ds(ic * TB * BC, TB * BC)],
                        XB[kb - NF8][:, ds(ic * TB * BC, TB * BC)],
                    )
            # lo half of W_ho streams last: its projections run in the shadow
            # of the final scan chain
            dma(whot_t[:, ds(0, 8 * O)], WHOT[:, ds(0, 8 * O)])  # g0..7
            dma(bias_t[:], BIAS)

            # --- header memsets for the fused scans ---
            for s in range(nseg):
                nc.gpsimd.memset(u0_seg(s)[:, :, ds(0, 2)], 1.0e30)
            nc.gpsimd.memset(u0_seg(0)[:, :, ds(2, 1)], 0.0)

            # --- PE warm-up: the HAM clock gate lifts a fixed ~5us after the
            # first matmul, so start that timer as early and cheaply as
            # possible (bf16: 1 cycle/col; fp32 warms cost 4x and queue
            # ahead of real work on the in-order PE) ---
            warm = consts.tile([128, TB * BC], bf16, tag="warm", name="warm")
            nc.gpsimd.memset(warm[:], 0.0)
            wps = gpool.tile([128, TB * BC], f32, tag="gp", name="warm_ps")
            NWARM = 2
            for i in range(NWARM):
                nc.tensor.matmul(
                    wps[:], warm[:, ds(0, 128)], warm[:],
                    start=(i == 0), stop=(i == NWARM - 1),
                )
            nc.scalar.copy(warm[:], wps[:])

            # --- output projection bookkeeping ---
            psy = fpool.tile([BC, O], f32, tag="fy", name="psy")
            proj_pending = list(range(NCH - 1, 0, -1))  # g15..g1; g0 last
            proj_done = [0]

            def emit_proj_g(g):
                nc.tensor.matmul(
                    psy[:], h_all[:, ds(g * BC, BC)], whot_t[:, ds(g * O, O)],
                    start=(proj_done[0] == 0), stop=(proj_done[0] == NCH - 1),
                )
                proj_done[0] += 1

            def emit_projs(n):
                for _ in range(min(n, len(proj_pending))):
                    emit_proj_g(proj_pending.pop(0))

            # --- small chunks: one GEMM unit per g.  g >= GFUSE (window WG,
            # block 15 only) write into the shared u12 tile and are scanned /
            # extracted by ONE fused instruction each at the end. ---
            for g in range(GFUSE, NCH):
                assert windows[g] == FW, (g, windows)
            nc.gpsimd.memset(
                u12_t[:].rearrange("p (x t) -> p x t", t=FW + 2)[:, :, ds(FW, 2)],
                1.0e30,
            )

            def emit_small_gemm(g):
                kg = windows[g]
                st0 = S - kg
                fb = st0 // TB
                toff = st0 % TB
                if g >= GFUSE:
                    u3 = u12_4d()[:, :, g - GFUSE]  # [p, b, WG+2]
                else:
                    u_t = usmall.tile(
                        [128, BC * (USMALL_W + 2)], f32, tag="us", name=f"u_g{g}"
                    )
                    u3 = u_t[:, ds(0, BC * (kg + 2))].rearrange(
                        "p (b t) -> p b t", b=BC
                    )
                    nc.gpsimd.memset(u3[:, :, ds(kg, 2)], 1.0e30)
                blocks = list(range(fb, NBLK))

                def t0_of(kb, fb=fb, toff=toff):
                    return toff if kb == fb else 0

                ps = {
                    kb: gpool.tile(
                        [128, (TB - t0_of(kb)) * BC], f32, tag="gp", name=f"gp_{g}_{kb}"
                    )
                    for kb in blocks
                }
                for ic in range(NI):
                    for kb in blocks:
                        t0 = t0_of(kb)
                        rhs = xbts[kb][:, ds(ic * TB * BC, TB * BC)]
                        out_ap = ps[kb][:]
                        if t0:
                            rhs = rhs.rearrange("p (b t) -> p b t", b=BC)[
                                :, :, ds(t0, TB - t0)
                            ]
                            out_ap = out_ap.rearrange("p (b t) -> p b t", b=BC)
                        nc.tensor.matmul(
                            out_ap,
                            wiht_t[:, ds(g * NI * 128 + ic * 128, 128)],
                            rhs,
                            start=(ic == 0),
                            stop=(ic == NI - 1),
                        )
                for kb in blocks:
                    t0 = t0_of(kb)
                    pos = kb * TB - st0 if kb > fb else 0
                    nc.scalar.copy(
                        u3[:, :, ds(pos, TB - t0)],
                        ps[kb][:].rearrange("p (b t) -> p b t", b=BC),
                    )
                return u3

            def emit_small(g):  # unfused path: GEMM + its own scan + extract
                kg = windows[g]
                u3 = emit_small_gemm(g)
                scn_g = (
                    scn_t[:, ds(int(offs[g - 1]), kg + 2)]
                    .rearrange("p (o t) -> p o t", o=1)
                    .broadcast_to([128, BC, kg + 2])
                )
                nc.vector._custom_dve(SCAN_OP, out=u3, in0=u3, in1=scn_g)
                nc.vector.tensor_copy(h_all[:, ds(g * BC, BC)], u3[:, :, kg - 1])

            def emit_small_f8(g):
                # fp8 DoubleRow GEMM for one fused chunk: X = block-15 tail
                # steps (x8s), weights from w8s; two j-passes of k=256
                c = g - GFUSE
                ps = gpool.tile([128, WG * BC], f32, tag="gp", name=f"gp8_{g}")
                for j in range(2):
                    w_ap = w8s_t[
                        :, ds(c * 4 * 128 + j * 2 * 128, 2 * 128)
                    ].rearrange("p (k h) -> p k h", k=2)
                    rhs = x8s_t[:, ds(j * 2 * BC * WG, 2 * BC * WG)].rearrange(
                        "p (k n) -> p k n", k=2
                    )
                    nc.tensor.matmul(
                        ps[:], w_ap, rhs,
                        start=(j == 0), stop=(j == 1), perf_mode=DR,
                    )
                nc.scalar.copy(
                    u12_4d()[:, :, c, ds(0, WG)],
                    ps[:].rearrange("p (b t) -> p b t", b=BC),
                )

            def emit_fused_small_scan():
                u3 = u12_t[:].rearrange(
                    "p (b ct) -> p b ct", b=BC
                )  # [p, b, NFU*(FW+2)]
                scn12 = (
                    scn_t[:, ds(int(offs[GFUSE - 1]), NFU * (FW + 2))]
                    .rearrange("p (o t) -> p o t", o=1)
                    .broadcast_to([128, BC, NFU * (FW + 2)])
                )
                nc.vector._custom_dve(SCAN_OP, out=u3, in0=u3, in1=scn12)
                # h extract for all fused chunks at once: dst is (c, b)-major,
                # so read the (b, c, t)-tile with c outer via strides
                src = u12_t[:].rearrange(
                    "p (b c t) -> p c b t", b=BC, c=NFU
                )[:, :, :, FW - 1]
                nc.vector.tensor_copy(h_all[:, ds(GFUSE * BC, NFU * BC)], src)

            # --- chunk 0: GEMM chases the DMA stream; fused per-seg scans ---
            seg_left = {s: nb for s, (_, nb) in enumerate(SEGS)}
            next_scan = [0]

            def copy_c0(kb, ps_t):
                s, i = seg_of[kb]
                nc.scalar.copy(
                    u0_seg(s)[:, :, ds(HDR + i * TB, TB)],
                    ps_t[:].rearrange("p (b t) -> p b t", b=BC),
                )
                seg_left[s] -= 1

            def emit_ready_scans():
                while next_scan[0] < nseg and seg_left[next_scan[0]] == 0:
                    s = next_scan[0]
                    w = seg_w[s]
                    if s > 0:
                        nc.vector.tensor_copy(
                            u0_seg(s)[:, :, 2], u0_seg(s - 1)[:, :, seg_w[s - 1] - 1]
                        )
                    scn_s = (
                        scn_t[:, ds(int(seg_c[s]), w)]
                        .rearrange("p (o t) -> p o t", o=1)
                        .broadcast_to([128, BC, w])
                    )
                    u3s = u0_seg(s)
                    nc.vector._custom_dve(SCAN_OP, out=u3s, in0=u3s, in1=scn_s)
                    next_scan[0] += 1

            def emit_c0_bf16_run(blocks):
                pss = {
                    kb: gpool.tile([128, TB * BC], f32, tag="gp", name=f"c0_{kb}")
                    for kb in blocks
                }
                for ic in range(NI):
                    for kb in blocks:
                        nc.tensor.matmul(
                            pss[kb][:],
                            wiht_t[:, ds(ic * 128, 128)],
                            xbts[kb][:, ds(ic * TB * BC, TB * BC)],
                            start=(ic == 0),
                            stop=(ic == NI - 1),
                        )
                for kb in blocks:
                    copy_c0(kb, pss[kb])

            def emit_c0_fp8_pair(k0):
                pair = (k0, k0 + 1)
                pss = {
                    kb: gpool.tile([128, TB * BC], f32, tag="gp", name=f"c8_{kb}")
                    for kb in pair
                }
                for j in range(2):
                    w_ap = w8_t[:].rearrange("p (j k h) -> p j k h", j=2, k=2)[:, j]
                    for kb in pair:
                        rhs = x8_half(kb).rearrange("p (j k n) -> p j k n", j=2, k=2)[
                            :, j
                        ]
                        nc.tensor.matmul(
                            pss[kb][:], w_ap, rhs,
                            start=(j == 0), stop=(j == 1), perf_mode=DR,
                        )
                for kb in pair:
                    copy_c0(kb, pss[kb])

            # --- interleaved schedule: the tiny fp8 small-chunk operands
            # arrive first, so all 12 fused small GEMMs run during the fp8
            # chunk-0 stream ramp; chunk-0 pairs chase the stream; bf16
            # blocks 10..13 trail; projections ride in DMA/scan shadows. ---
            for g in range(NCH - 1, GFUSE - 1, -1):
                emit_small_f8(g)
            emit_fused_small_scan()  # one scan + one extract for g4..g15
            emit_c0_fp8_pair(0)
            emit_ready_scans()
            emit_c0_fp8_pair(2)
            emit_ready_scans()
            emit_c0_fp8_pair(4)
            emit_ready_scans()
            emit_small(3)
            emit_small(2)
            emit_c0_fp8_pair(6)
            emit_ready_scans()
            emit_small(1)
            emit_c0_fp8_pair(8)
            emit_ready_scans()
            emit_c0_bf16_run([14, 15])  # resident early; tail-seg prefill
            emit_projs(4)  # g15..g12 (WHOT hi half)
            emit_c0_bf16_run([10])
            emit_ready_scans()
            emit_projs(2)
            emit_c0_bf16_run([11])
            emit_ready_scans()
            emit_projs(2)
            emit_c0_bf16_run([12])
            emit_ready_scans()
            emit_c0_bf16_run([13])
            emit_ready_scans()
            # remaining g-projections run in the shadow of the final segment
            # scans, fed by the late-arriving lo half of W_ho
            emit_projs(len(proj_pending))
            assert next_scan[0] == nseg and not any(seg_left.values()), (
                next_scan, seg_left,
            )
            # chunk-0 h extract, then its projection closes the accumulation
            nc.vector.tensor_copy(
                h_all[:, ds(0, BC)], u0_seg(nseg - 1)[:, :, seg_w[nseg - 1] - 1]
            )
            emit_proj_g(0)
            assert proj_done[0] == NCH

            y_t = ypool.tile([BC, O], f32, tag="y", name="y_t")
            nc.vector.tensor_tensor(y_t[:], psy[:], bias_t[:], mybir.AluOpType.add)
            nc.sync.dma_start(Y, y_t[:])
    nc.compile()
    return nc


def _get_program(windows):
    key = (windows, os.environ.get("DIAG_LN"), os.environ.get("DIAG_FP8BLK"))
    if key not in _CACHE:
        _CACHE[key] = _build(windows)
    return _CACHE[key]


def _ensure_ntff_hook():
    """Provide antenv.axon_hooks (absent in this image) so trace=True works."""
    import sys
    import types

    if "antenv.axon_hooks" in sys.modules:
        return True
    try:
        import antenv

        mod = types.ModuleType("antenv.axon_hooks")
        mod._hook = None

        def set_axon_ntff_profile_hook(h):
            mod._hook = h

        def get_axon_ntff_profile_hook():
            return mod._hook

        mod.set_axon_ntff_profile_hook = set_axon_ntff_profile_hook
        mod.get_axon_ntff_profile_hook = get_axon_ntff_profile_hook
        sys.modules["antenv.axon_hooks"] = mod
        antenv.axon_hooks = mod

        from trn_agent_boot.trn_boot import _ntff_profile_via_ctypes

        hook = _ntff_profile_via_ctypes("/opt/axon/libaxon_pjrt.so")
        mod.set_axon_ntff_profile_hook(hook)
        return hook is not None
    except Exception:
        return False


def kernel(X, W_ih, hh, W_ho, b_ho):
    import ml_dtypes

    from concourse import bass_utils

    X = np.asarray(X, dtype=np.float32)
    W_ih = np.asarray(W_ih, dtype=np.float32)
    hh = np.asarray(hh, dtype=np.float32)
    W_ho = np.asarray(W_ho, dtype=np.float32)
    b_ho = np.asarray(b_ho, dtype=np.float32)

    plan = _make_plan(hh)
    perm = plan["perm"]
    nc = _get_program(plan["windows"])
    NF8 = _fp8_nblk()

    bf = ml_dtypes.bfloat16
    f8 = ml_dtypes.float8_e4m3
    # WIHT [128, 4*NI*128]: line p = [g, ic, hsub] for chunks 0..3 only
    wiht = np.ascontiguousarray(
        W_ih[perm[: 4 * 128]]
        .T.reshape(NI, 128, 4, 128)
        .transpose(1, 2, 0, 3)
        .reshape(128, -1)
    ).astype(bf)
    # W8S [128, (c, j, k, h)]: fp8 weights for fused chunks g in [GFUSE, 16)
    w8s = np.ascontiguousarray(
        W_ih[perm[GFUSE * 128 :]]
        .reshape(NCH - GFUSE, 128, 2, 2, 128)  # [c, h, j, k, p]
        .transpose(4, 0, 2, 3, 1)
        .reshape(128, -1)
    ).astype(f8)
    # WHOT [128, NCH*O]: line p = [g, o] with value W_ho[o, h=g*128+p]
    whot = np.ascontiguousarray(
        W_ho[:, perm].T.reshape(NCH, 128, O).transpose(1, 0, 2).reshape(128, NCH * O)
    ).astype(bf)
    bias = np.tile(b_ho[None, :], (BC, 1)).astype(np.float32)

    common = {
        "WIHT": wiht,
        "W8S": w8s,
        "WHOT": whot,
        "BIAS": bias,
        "SCN": plan["SCN"].astype(bf),
    }
    if NF8:
        # W8 [128, (j,k,h)]: chunk-0 weights, i = (2j+k)*128 + p
        w0 = W_ih[perm[:128]]  # [128h, I]
        common["W8"] = np.ascontiguousarray(
            w0.T.reshape(2, 2, 128, 128).transpose(2, 0, 1, 3).reshape(128, 512)
        ).astype(f8)
    in_maps = []
    for m in range(NCORES):
        im = dict(common)
        xm = X[:, m * BC : (m + 1) * BC, :]  # [S, BC, I]
        # bf16 blocks: [128(i%128), (ic, b, tau)]
        xt = xm.transpose(2, 1, 0).reshape(NI, 128, BC, NBLK, TB)
        xt = np.ascontiguousarray(xt.transpose(3, 1, 0, 2, 4)).reshape(
            NBLK, 128, NI * BC * TB
        )
        im["XB"] = xt[NF8:].astype(bf)
        # X8S [128, (j, k, b, tau)]: fp8 copy of the last WG steps (block 15
        # tail) feeding the fused small-chunk GEMMs
        im["X8S"] = np.ascontiguousarray(
            xm[S - WG :]
            .transpose(2, 1, 0)  # [i, b, tau]
            .reshape(2, 2, 128, BC, WG)  # [j, k, p, b, tau]
            .transpose(2, 0, 1, 3, 4)
            .reshape(128, 4 * BC * WG)
        ).astype(f8)
        if NF8:
            x8 = np.empty((NF8, 128, 4 * BC * TB), dtype=f8)
            for kb in range(NF8):
                blk = xm[kb * TB : (kb + 1) * TB]  # [TB, BC, I]
                a = blk.transpose(2, 1, 0).reshape(2, 2, 128, BC, TB)  # [j,k,p,b,t]
                x8[kb] = (
                    np.ascontiguousarray(a.transpose(2, 0, 1, 3, 4))
                    .reshape(128, 4 * BC * TB)
                    .astype(f8)
                )
            # pack block pairs along the line dim: [NF8//2, 128, 8*BC*TB]
            im["X8"] = np.ascontiguousarray(
                x8.reshape(NF8 // 2, 2, 128, 4 * BC * TB).transpose(0, 2, 1, 3)
            ).reshape(NF8 // 2, 128, 8 * BC * TB)
        in_maps.append(im)

    trace = bool(int(os.environ.get("DIAG_TRACE", "0")))
    if trace:
        trace = _ensure_ntff_hook()
    res = None
    for attempt in range(3):
        try:
            res = bass_utils.run_bass_kernel_spmd(
                nc,
                in_maps,
                core_ids=list(range(NCORES)),
                trace=trace,
                tmpdir=os.environ.get("DIAG_TRACE_DIR") or None,
            )
            break
        except Exception:
            if attempt == 2:
                raise
            trace = False  # retry without profiling
    if res.exec_time_ns is not None:
        kernel.last_exec_time_ns = res.exec_time_ns
        kernel.last_mean_exec_time_ns = res.mean_exec_time_ns
    Yfull = np.concatenate([r["Y"] for r in res.results], axis=0)
    return Yfull


kernel.last_exec_time_ns = None
kernel.last_mean_exec_time_ns = None


# revision 63
# speedup vs baseline: 1.0606x; 1.0286x over previous
"""Trainium2 Bass kernel for nn_Diagnet (S=1024, B=64, I=512, H=2048, O=512).

    u = einsum('sbi,hi->sbh', X, W_ih)
    h_t = |u_t + hh * h_{t-1}|   (scan over S, only final h needed)
    Y = h_final @ W_ho.T + b_ho

Strategy (8 NeuronCores, data-parallel over batch, BC=8 rows per core):

* H lanes are permuted so hh is sorted descending and split into 16
  chunks of 128.  A chunk whose largest decay a satisfies a^K < tol
  only needs the last K steps, so each chunk gets a window K_g and the
  GEMM + scan skip everything earlier.  Chunk 0 keeps the full 1024.
* The recurrence is a custom DVE instruction that folds a window in
  one go: m[t] = |m[t-1] - u[t]*scn[t]| with scn[t] = -a^(K-1-t)
  (prescale folds the decay into the stream; the minus sign turns
  ABSOLUTE_DIFF into abs-add).  h_final = last element.
* All 8 batch rows fold in ONE scan instruction per segment via a
  3-column header per row: a BIG separator pair scaled (-1, +1)
  absorbs and exactly zeroes the running state between rows, then a
  seed column (scn=-1) re-injects that row's carry from the previous
  segment (h >= 0 so |0 - s*(-1)| = s).  Seed values are copied
  between segments by a tiny DVE copy, keeping the serial chain on
  one engine.
* Chunk-0 blocks t<640 run in fp8 (e4m3) with DoubleRow perf mode
  (256-deep contraction, half the matmul passes and half the X bytes);
  late blocks and everything else stay bf16.  Decay weighting keeps
  the fp8 quantization error ~1.5% of max|Y| (gate 2e-2).
* Small chunks g>=4 also run in fp8 (their h decay fast; g1..g3 carry
  too much output mass and stay bf16): one fused DoubleRow GEMM per
  chunk off a tiny fp8 copy of the block-15 tail, one fused scan and
  one fused h-extract for all twelve.
* Two HWDGE queues stream the inputs: the scalar queue prefetches the
  small fp8 operands + first chunk-0 pair (it clears its preamble
  ~2us before sync), while the sync queue streams, in priority order:
  X15, SCN, remaining fp8 X pairs, bf16 W (chunks 0..3), W_ho hi half,
  bf16 X blocks 10..13, W_ho lo half (its projections run in the final
  scan chain's shadow).  The PE chases arrivals; the 16 output
  projections interleave into DMA slack instead of trailing at the end.
"""

import math
import os

from contextlib import ExitStack

import numpy as np

S, B, I, H, O = 1024, 64, 512, 2048, 512
NCORES = 8
BC = B // NCORES  # 8 batch rows per core
TB = 64  # X block granularity
WG = 32  # truncation-window granularity
NBLK = S // TB  # 16
NCH = H // 128  # 16 h-chunks
NI = I // 128  # 4 i-chunks
USMALL_W = 256  # max window (cols) for chunks g>=1
# chunk-0 scan segments as (first_block, n_blocks); full coverage of 0..15.
# Blocks 12/13 arrive last, so they get 1-block segments to shorten the
# serial scan tail; 14/15 (resident early for the small chunks) scan last.
SEGS = [(0, 2), (2, 2), (4, 2), (6, 2), (8, 2), (10, 2), (12, 1), (13, 3)]
GFUSE = 4  # small chunks g >= GFUSE share one u tile / one fused scan
FW = 32  # fused-chunk window (these chunks all truncate to one half-block)
HDR = 3  # per-row header cols: BIG sep (-1), BIG sep (+1), seed (-1)

_CACHE = {}


def _fp8_nblk():
    n = int(os.environ.get("DIAG_FP8BLK", "10"))
    assert n % 2 == 0 and 0 <= n <= (S - USMALL_W) // TB, n
    return n


def _seg_meta():
    """Per-segment (start_col, width) in per-row cols, and block->seg map."""
    seg_w = [HDR + nb * TB for _, nb in SEGS]
    seg_c = np.concatenate([[0], np.cumsum(seg_w)]).astype(int)
    seg_of = {}
    for s, (fb, nb) in enumerate(SEGS):
        for i in range(nb):
            seg_of[fb + i] = (s, i)
    return seg_w, seg_c, seg_of  # widths, col starts (len nseg+1), block map


def _register_scan_ops():
    """Fold op: m[t] = |m[t-1] - in0[t]*in1[t]|, zero-initialized."""
    import concourse.dve_ops as dve_ops
    from concourse.dve_spec import C0, Spec, Src0, Src1, Zero, scan, lower, AluOp
    from concourse.dve_uop import DveOpSpec

    have = {op.name: op for op in dve_ops.OPS}
    if "ABSDIFF_SCALE_SCAN_ANT" in have:
        return have["ABSDIFF_SCALE_SCAN_ANT"]

    def _ref(in0, in1, s0, s1, imm2):
        x = in0.astype(np.float32) * in1.astype(np.float32)
        out = np.empty_like(x)
        m = np.zeros(x.shape[0], np.float32)
        for t in range(x.shape[1]):
            m = np.abs(m - x[:, t])
            out[:, t] = m
        return out

    spec = Spec(
        body=scan(AluOp.ABSOLUTE_DIFF, Src0 * Src1, init=Zero),
        reference=_ref,
    )
    row = max(dve_ops._SUB_OPCODE_FOR_NAME.values()) + 1
    assert row < 0x20
    shas = {}
    for ver in ("v3", "v4"):
        s = DveOpSpec(
            name="ABSDIFF_SCALE_SCAN_ANT", opcode=row, uops=lower(spec, ver=ver),
            rd1_en=True,
        )
        shas[ver] = s.sha(ver)
    op = dve_ops.DveOp("ABSDIFF_SCALE_SCAN_ANT", spec, subdim=False, uops_sha=shas)
    dve_ops._SUB_OPCODE_FOR_NAME["ABSDIFF_SCALE_SCAN_ANT"] = row
    dve_ops.OPS.append(op)
    dve_ops.CUSTOM_DVE_SPECS["ABSDIFF_SCALE_SCAN_ANT"] = spec
    return op


def _windows(hh):
    ln = float(os.environ.get("DIAG_LN", "7.0"))
    a = np.maximum(np.abs(hh.astype(np.float64)), 1e-30)
    perm = np.argsort(-a, kind="stable")
    ag = a[perm].reshape(NCH, 128)
    windows = []
    for g in range(NCH):
        amax = ag[g, 0]
        if S * math.log(amax) >= -ln:
            kg = S
        else:
            kg = int(math.ceil(ln / math.log(1.0 / amax)))
        kg = min(S, max(WG, ((kg + WG - 1) // WG) * WG))
        windows.append(kg)
    assert windows[0] == S, windows
    assert all(windows[g] >= windows[g + 1] for g in range(NCH - 1)), windows
    assert all(k <= USMALL_W for k in windows[1:]), (windows, "raise USMALL_W")
    return perm, ag, tuple(windows)


def _small_offs(windows, base):
    """Start col of each small chunk's scn piece (g>=1), after chunk-0 base."""
    widths = [0] + [windows[g] + 2 for g in range(1, NCH)]
    return base + np.cumsum(widths).astype(int)  # index by g-1 ... use [g-1]


def _make_plan(hh):
    perm, ag, windows = _windows(hh)
    seg_w, seg_c, _ = _seg_meta()
    u0w = int(seg_c[-1])  # per-row cols of chunk-0 stream (1045)
    offs = np.concatenate(
        [[u0w], u0w + np.cumsum([windows[g] + 2 for g in range(1, NCH)])]
    ).astype(int)
    total = int(offs[-1])
    scn = np.zeros((128, total), dtype=np.float64)
    a0 = ag[0]
    col = 0
    for (fb, nb) in SEGS:
        scn[:, col] = -1.0
        scn[:, col + 1] = 1.0
        scn[:, col + 2] = -1.0
        t = np.arange(fb * TB, (fb + nb) * TB)
        scn[:, col + HDR : col + HDR + nb * TB] = -(
            a0[:, None] ** (S - 1 - t)[None, :]
        )
        col += HDR + nb * TB
    assert col == u0w
    for g in range(1, NCH):
        kg = windows[g]
        off = int(offs[g - 1])
        tau = np.arange(kg)
        scn[:, off : off + kg] = -(ag[g][:, None] ** (kg - 1 - tau)[None, :])
        scn[:, off + kg] = -1.0
        scn[:, off + kg + 1] = 1.0
    return {"perm": perm, "windows": windows, "offs": offs, "SCN": scn}


def _build(windows):
    import concourse.mybir as mybir
    import concourse.tile as tile
    from concourse import bacc
    from concourse.bass import ds

    SCAN_OP = _register_scan_ops()
    f32 = mybir.dt.float32
    bf16 = mybir.dt.bfloat16
    f8 = mybir.dt.float8e4
    DR = mybir.MatmulPerfMode.DoubleRow
    NF8 = _fp8_nblk()

    seg_w, seg_c, seg_of = _seg_meta()
    u0w = int(seg_c[-1])
    offs = np.concatenate(
        [[u0w], u0w + np.cumsum([windows[g] + 2 for g in range(1, NCH)])]
    ).astype(int)
    total_scn = int(offs[-1])
    nseg = len(SEGS)

    nc = bacc.Bacc("TRN2", target_bir_lowering=False, debug=False, num_devices=NCORES)
    XB = nc.dram_tensor(
        "XB", [NBLK - NF8, 128, NI * TB * BC], bf16, kind="ExternalInput"
    ).ap()  # blocks NF8..15, line [ic, b, tau]
    if NF8:
        # blocks 0..NF8-1 packed in pairs; per block the line is
        # [j, k, b, tau] with i = (2j+k)*128+p.  Pairing keeps 4KB DMA lines.
        X8 = nc.dram_tensor(
            "X8", [NF8 // 2, 128, 8 * TB * BC], f8, kind="ExternalInput"
        ).ap()
        W8 = nc.dram_tensor("W8", [128, 4 * 128], f8, kind="ExternalInput").ap()
    # fp8 operands for the fused small chunks: block-15 tail steps of X and
    # the g4..g15 weight rows, both in DoubleRow [j, k, ...] layout
    NFU_B = NCH - GFUSE
    X8S = nc.dram_tensor("X8S", [128, 4 * BC * WG], f8, kind="ExternalInput").ap()
    W8S = nc.dram_tensor("W8S", [128, NFU_B * 4 * 128], f8, kind="ExternalInput").ap()
    # bf16 input weights only needed for chunks 0..3 now
    WIHT = nc.dram_tensor("WIHT", [128, 4 * NI * 128], bf16, kind="ExternalInput").ap()
    WHOT = nc.dram_tensor("WHOT", [128, NCH * O], bf16, kind="ExternalInput").ap()
    SCN = nc.dram_tensor("SCN", [128, total_scn], bf16, kind="ExternalInput").ap()
    BIAS = nc.dram_tensor("BIAS", [BC, O], f32, kind="ExternalInput").ap()
    Y = nc.dram_tensor("Y", [BC, O], f32, kind="ExternalOutput").ap()

    with tile.TileContext(nc) as tc:
        with ExitStack() as ctx:
            consts = ctx.enter_context(tc.tile_pool(name="consts", bufs=1))
            xpool = ctx.enter_context(tc.tile_pool(name="xt", bufs=1))
            ubig = ctx.enter_context(tc.tile_pool(name="ubig", bufs=1))
            usmall = ctx.enter_context(tc.tile_pool(name="usmall", bufs=4))
            ypool = ctx.enter_context(tc.tile_pool(name="yout", bufs=1))
            gpool = ctx.enter_context(tc.tile_pool(name="gpsum", bufs=7, space="PSUM"))
            fpool = ctx.enter_context(tc.tile_pool(name="fpsum", bufs=1, space="PSUM"))

            wiht_t = consts.tile([128, 4 * NI * 128], bf16, tag="wiht", name="wiht_t")
            w8s_t = consts.tile([128, (NCH - GFUSE) * 4 * 128], f8, tag="w8s", name="w8s_t")
            x8s_t = consts.tile([128, 4 * BC * WG], f8, tag="x8s", name="x8s_t")
            whot_t = consts.tile([128, NCH * O], bf16, tag="whot", name="whot_t")
            scn_t = consts.tile([128, total_scn], bf16, tag="scn", name="scn_t")
            bias_t = ypool.tile([BC, O], f32, tag="bias", name="bias_t")
            h_all = consts.tile([128, NCH * BC], bf16, tag="hall", name="h_all")
            if NF8:
                w8_t = consts.tile([128, 4 * 128], f8, tag="w8", name="w8_t")
            # fp8 X packed two blocks per tile (4KB DMA lines for full rate)
            x8pts = [
                xpool.tile([128, 8 * TB * BC], f8, tag=f"x8p_{pi}", name=f"x8p_{pi}")[:]
                for pi in range(NF8 // 2)
            ]

            def x8_half(kb):  # [p, (j,k,b,tau)] slice for one block
                return x8pts[kb // 2][:, ds((kb % 2) * 4 * TB * BC, 4 * TB * BC)]

            xbts = {
                kb: xpool.tile(
                    [128, NI * TB * BC], bf16, tag=f"xb_{kb}", name=f"xb_{kb}"
                )[:]
                for kb in range(NF8, NBLK)
            }
            u0_t = ubig.tile([128, BC * u0w], f32, tag="u0", name="u0")
            # fused u tile for small chunks g in [GFUSE, 16): [p, (b, c, FW+2)].
            # b-major so the fused scan is rank-3: [p, b, (c t)] with the scn
            # stream broadcast over b; each (b, c) sub-stream is independent.
            NFU = NCH - GFUSE
            u12_t = ubig.tile(
                [128, BC * NFU * (FW + 2)], f32, tag="u12", name="u12"
            )

            def u12_4d():  # [p, b, c, t]
                return u12_t[:].rearrange(
                    "p (b c t) -> p b c t", b=BC, c=NFU
                )

            def u0_seg(s):  # [p, b, width_s]
                return u0_t[:, ds(int(seg_c[s]) * BC, BC * seg_w[s])].rearrange(
                    "p (b t) -> p b t", b=BC
                )

            # --- DMA stream (single HWDGE queue; order = priority).
            # fp8 chunk-0 blocks go first so the GEMM->scan chain chases the
            # stream from the start; small-chunk weights and X14/15 interleave
            # to fill PE slack; chunk-0-only bf16 blocks 10..13 stream last. ---
            def wp(g0, ng):
                return ds(g0 * NI * 128, ng * NI * 128)

            def xbd(kb):
                nc.sync.dma_start(xbts[kb], XB[kb - NF8])

            dma = nc.sync.dma_start
            assert NF8 == 10, "stream schedule is tuned for DIAG_FP8BLK=10"
            # Prefetch the small-chunk fp8 operands + first chunk-0 pair on
            # the scalar HWDGE queue: it clears its preamble ~2us before sync
            # and doubles early DMA concurrency through the ramp.  A single
            # in-flight DMA tops out well below the aggregate rate, so W8S is
            # split in two.
            nc.scalar.dma_start(w8s_t[:], W8S)
            nc.scalar.dma_start(x8s_t[:], X8S)
            nc.scalar.dma_start(w8_t[:], W8)
            nc.scalar.dma_start(x8pts[0], X8[0])
            xbd(15)
            dma(scn_t[:], SCN)
            dma(x8pts[1], X8[1])
            xbd(14)
            dma(x8pts[2], X8[2])
            dma(x8pts[3], X8[3])
            dma(wiht_t[:], WIHT)  # bf16 weights for chunks 0..3
            dma(x8pts[4], X8[4])
            dma(whot_t[:, ds(8 * O, 8 * O)], WHOT[:, ds(8 * O, 8 * O)])  # g8..15
            xbd(10)
            xbd(11)
            xbd(12)
            xbd(13)
            # lo half of W_ho streams last: its projections run in the shadow
            # of the final scan chain, while XB12/13 (which gate that chain)
            # arrive earlier
            dma(whot_t[:, ds(0, 8 * O)], WHOT[:, ds(0, 8 * O)])  # g0..7
            dma(bias_t[:], BIAS)

            # --- header memsets for the fused scans ---
            for s in range(nseg):
                nc.gpsimd.memset(u0_seg(s)[:, :, ds(0, 2)], 1.0e30)
            nc.gpsimd.memset(u0_seg(0)[:, :, ds(2, 1)], 0.0)

            # --- PE warm-up: the HAM clock gate lifts a fixed ~5us after the
            # first matmul, so start that timer as early and cheaply as
            # possible (bf16: 1 cycle/col; fp32 warms cost 4x and queue
            # ahead of real work on the in-order PE) ---
            warm = consts.tile([128, TB * BC], bf16, tag="warm", name="warm")
            nc.gpsimd.memset(warm[:], 0.0)
            wps = gpool.tile([128, TB * BC], f32, tag="gp", name="warm_ps")
            NWARM = 2
            for i in range(NWARM):
                nc.tensor.matmul(
                    wps[:], warm[:, ds(0, 128)], warm[:],
                    start=(i == 0), stop=(i == NWARM - 1),
                )
            nc.scalar.copy(warm[:], wps[:])

            # --- output projection bookkeeping ---
            psy = fpool.tile([BC, O], f32, tag="fy", name="psy")
            proj_pending = list(range(NCH - 1, 0, -1))  # g15..g1; g0 last
            proj_done = [0]

            def emit_proj_g(g):
                nc.tensor.matmul(
                    psy[:], h_all[:, ds(g * BC, BC)], whot_t[:, ds(g * O, O)],
                    start=(proj_done[0] == 0), stop=(proj_done[0] == NCH - 1),
                )
                proj_done[0] += 1

            def emit_projs(n):
                for _ in range(min(n, len(proj_pending))):
                    emit_proj_g(proj_pending.pop(0))

            # --- small chunks: one GEMM unit per g.  g >= GFUSE (window WG,
            # block 15 only) write into the shared u12 tile and are scanned /
            # extracted by ONE fused instruction each at the end. ---
            for g in range(GFUSE, NCH):
                assert windows[g] == FW, (g, windows)
            nc.gpsimd.memset(
                u12_t[:].rearrange("p (x t) -> p x t", t=FW + 2)[:, :, ds(FW, 2)],
                1.0e30,
            )

            def emit_small_gemm(g):
                kg = windows[g]
                st0 = S - kg
                fb = st0 // TB
                toff = st0 % TB
                if g >= GFUSE:
                    u3 = u12_4d()[:, :, g - GFUSE]  # [p, b, WG+2]
                else:
                    u_t = usmall.tile(
                        [128, BC * (USMALL_W + 2)], f32, tag="us", name=f"u_g{g}"
                    )
                    u3 = u_t[:, ds(0, BC * (kg + 2))].rearrange(
                        "p (b t) -> p b t", b=BC
                    )
                    nc.gpsimd.memset(u3[:, :, ds(kg, 2)], 1.0e30)
                blocks = list(range(fb, NBLK))

                def t0_of(kb, fb=fb, toff=toff):
                    return toff if kb == fb else 0

                ps = {
                    kb: gpool.tile(
                        [128, (TB - t0_of(kb)) * BC], f32, tag="gp", name=f"gp_{g}_{kb}"
                    )
                    for kb in blocks
                }
                for ic in range(NI):
                    for kb in blocks:
                        t0 = t0_of(kb)
                        rhs = xbts[kb][:, ds(ic * TB * BC, TB * BC)]
                        out_ap = ps[kb][:]
                        if t0:
                            rhs = rhs.rearrange("p (b t) -> p b t", b=BC)[
                                :, :, ds(t0, TB - t0)
                            ]
                            out_ap = out_ap.rearrange("p (b t) -> p b t", b=BC)
                        nc.tensor.matmul(
                            out_ap,
                            wiht_t[:, ds(g * NI * 128 + ic * 128, 128)],
                            rhs,
                            start=(ic == 0),
                            stop=(ic == NI - 1),
                        )
                for kb in blocks:
                    t0 = t0_of(kb)
                    pos = kb * TB - st0 if kb > fb else 0
                    nc.scalar.copy(
                        u3[:, :, ds(pos, TB - t0)],
                        ps[kb][:].rearrange("p (b t) -> p b t", b=BC),
                    )
                return u3

            def emit_small(g):  # unfused path: GEMM + its own scan + extract
                kg = windows[g]
                u3 = emit_small_gemm(g)
                scn_g = (
                    scn_t[:, ds(int(offs[g - 1]), kg + 2)]
                    .rearrange("p (o t) -> p o t", o=1)
                    .broadcast_to([128, BC, kg + 2])
                )
                nc.vector._custom_dve(SCAN_OP, out=u3, in0=u3, in1=scn_g)
                nc.vector.tensor_copy(h_all[:, ds(g * BC, BC)], u3[:, :, kg - 1])

            def emit_small_f8(g):
                # fp8 DoubleRow GEMM for one fused chunk: X = block-15 tail
                # steps (x8s), weights from w8s; two j-passes of k=256
                c = g - GFUSE
                ps = gpool.tile([128, WG * BC], f32, tag="gp", name=f"gp8_{g}")
                for j in range(2):
                    w_ap = w8s_t[
                        :, ds(c * 4 * 128 + j * 2 * 128, 2 * 128)
                    ].rearrange("p (k h) -> p k h", k=2)
                    rhs = x8s_t[:, ds(j * 2 * BC * WG, 2 * BC * WG)].rearrange(
                        "p (k n) -> p k n", k=2
                    )
                    nc.tensor.matmul(
                        ps[:], w_ap, rhs,
                        start=(j == 0), stop=(j == 1), perf_mode=DR,
                    )
                nc.scalar.copy(
                    u12_4d()[:, :, c, ds(0, WG)],
                    ps[:].rearrange("p (b t) -> p b t", b=BC),
                )

            def emit_fused_small_scan():
                u3 = u12_t[:].rearrange(
                    "p (b ct) -> p b ct", b=BC
                )  # [p, b, NFU*(FW+2)]
                scn12 = (
                    scn_t[:, ds(int(offs[GFUSE - 1]), NFU * (FW + 2))]
                    .rearrange("p (o t) -> p o t", o=1)
                    .broadcast_to([128, BC, NFU * (FW + 2)])
                )
                nc.vector._custom_dve(SCAN_OP, out=u3, in0=u3, in1=scn12)
                # h extract for all fused chunks at once: dst is (c, b)-major,
                # so read the (b, c, t)-tile with c outer via strides
                src = u12_t[:].rearrange(
                    "p (b c t) -> p c b t", b=BC, c=NFU
                )[:, :, :, FW - 1]
                nc.vector.tensor_copy(h_all[:, ds(GFUSE * BC, NFU * BC)], src)

            # --- chunk 0: GEMM chases the DMA stream; fused per-seg scans ---
            seg_left = {s: nb for s, (_, nb) in enumerate(SEGS)}
            next_scan = [0]

            def copy_c0(kb, ps_t):
                s, i = seg_of[kb]
                nc.scalar.copy(
                    u0_seg(s)[:, :, ds(HDR + i * TB, TB)],
                    ps_t[:].rearrange("p (b t) -> p b t", b=BC),
                )
                seg_left[s] -= 1

            def emit_ready_scans():
                while next_scan[0] < nseg and seg_left[next_scan[0]] == 0:
                    s = next_scan[0]
                    w = seg_w[s]
                    if s > 0:
                        nc.vector.tensor_copy(
                            u0_seg(s)[:, :, 2], u0_seg(s - 1)[:, :, seg_w[s - 1] - 1]
                        )
                    scn_s = (
                        scn_t[:, ds(int(seg_c[s]), w)]
                        .rearrange("p (o t) -> p o t", o=1)
                        .broadcast_to([128, BC, w])
                    )
                    u3s = u0_seg(s)
                    nc.vector._custom_dve(SCAN_OP, out=u3s, in0=u3s, in1=scn_s)
                    next_scan[0] += 1

            def emit_c0_bf16_run(blocks):
                pss = {
                    kb: gpool.tile([128, TB * BC], f32, tag="gp", name=f"c0_{kb}")
                    for kb in blocks
                }
                for ic in range(NI):
                    for kb in blocks:
                        nc.tensor.matmul(
                            pss[kb][:],
                            wiht_t[:, ds(ic * 128, 128)],
                            xbts[kb][:, ds(ic * TB * BC, TB * BC)],
                            start=(ic == 0),
                            stop=(ic == NI - 1),
                        )
                for kb in blocks:
                    copy_c0(kb, pss[kb])

            def emit_c0_fp8_pair(k0):
                pair = (k0, k0 + 1)
                pss = {
                    kb: gpool.tile([128, TB * BC], f32, tag="gp", name=f"c8_{kb}")
                    for kb in pair
                }
                for j in range(2):
                    w_ap = w8_t[:].rearrange("p (j k h) -> p j k h", j=2, k=2)[:, j]
                    for kb in pair:
                        rhs = x8_half(kb).rearrange("p (j k n) -> p j k n", j=2, k=2)[
                            :, j
                        ]
                        nc.tensor.matmul(
                            pss[kb][:], w_ap, rhs,
                            start=(j == 0), stop=(j == 1), perf_mode=DR,
                        )
                for kb in pair:
                    copy_c0(kb, pss[kb])

            # --- interleaved schedule: the tiny fp8 small-chunk operands
            # arrive first, so all 12 fused small GEMMs run during the fp8
            # chunk-0 stream ramp; chunk-0 pairs chase the stream; bf16
            # blocks 10..13 trail; projections ride in DMA/scan shadows. ---
            for g in range(NCH - 1, GFUSE - 1, -1):
                emit_small_f8(g)
            emit_fused_small_scan()  # one scan + one extract for g4..g15
            emit_c0_fp8_pair(0)
            emit_ready_scans()
            emit_c0_fp8_pair(2)
            emit_ready_scans()
            emit_c0_fp8_pair(4)
            emit_ready_scans()
            emit_small(3)
            emit_small(2)
            emit_c0_fp8_pair(6)
            emit_ready_scans()
            emit_small(1)
            emit_c0_fp8_pair(8)
            emit_ready_scans()
            emit_c0_bf16_run([14, 15])  # resident early; tail-seg prefill
            emit_projs(4)  # g15..g12 (WHOT hi half)
            emit_c0_bf16_run([10])
            emit_ready_scans()
            emit_projs(2)
            emit_c0_bf16_run([11])
            emit_ready_scans()
            emit_projs(2)
            emit_c0_bf16_run([12])
            emit_ready_scans()
            emit_c0_bf16_run([13])
            emit_ready_scans()
            # remaining g-projections run in the shadow of the final segment
            # scans, fed by the late-arriving lo half of W_ho
            emit_projs(len(proj_pending))
            assert next_scan[0] == nseg and not any(seg_left.values()), (
                next_scan, seg_left,
            )
            # chunk-0 h extract, then its projection closes the accumulation
            nc.vector.tensor_copy(
                h_all[:, ds(0, BC)], u0_seg(nseg - 1)[:, :, seg_w[nseg - 1] - 1]
            )
            emit_proj_g(0)
            assert proj_done[0] == NCH

            y_t = ypool.tile([BC, O], f32, tag="y", name="y_t")
            nc.vector.tensor_tensor(y_t[:], psy[:], bias_t[:], mybir.AluOpType.add)
            nc.sync.dma_start(Y, y_t[:])
    nc.compile()
    return nc


def _get_program(windows):
    key = (windows, os.environ.get("DIAG_LN"), os.environ.get("DIAG_FP8BLK"))
    if key not in _CACHE:
        _CACHE[key] = _build(windows)
    return _CACHE[key]


def _ensure_ntff_hook():
    """Provide antenv.axon_hooks (absent in this image) so trace=True works."""
    import sys
    import types

    if "antenv.axon_hooks" in sys.modules:
        return True
    try:
        import antenv

        mod = types.ModuleType("antenv.axon_hooks")
        mod._hook = None

        def set_axon_ntff_profile_hook(h):
            mod._hook = h

        def get_axon_ntff_profile_hook():
            return mod._hook

        mod.set_axon_ntff_profile_hook = set_axon_ntff_profile_hook
        mod.get_axon_ntff_profile_hook = get_axon_ntff_profile_hook
        sys.modules["antenv.axon_hooks"] = mod
        antenv.axon_hooks = mod

        from trn_agent_boot.trn_boot import _ntff_profile_via_ctypes

        hook = _ntff_profile_via_ctypes("/opt/axon/libaxon_pjrt.so")
        mod.set_axon_ntff_profile_hook(hook)
        return hook is not None
    except Exception:
        return False


def kernel(X, W_ih, hh, W_ho, b_ho):
    import ml_dtypes

    from concourse import bass_utils

    X = np.asarray(X, dtype=np.float32)
    W_ih = np.asarray(W_ih, dtype=np.float32)
    hh = np.asarray(hh, dtype=np.float32)
    W_ho = np.asarray(W_ho, dtype=np.float32)
    b_ho = np.asarray(b_ho, dtype=np.float32)

    plan = _make_plan(hh)
    perm = plan["perm"]
    nc = _get_program(plan["windows"])
    NF8 = _fp8_nblk()

    bf = ml_dtypes.bfloat16
    f8 = ml_dtypes.float8_e4m3
    # WIHT [128, 4*NI*128]: line p = [g, ic, hsub] for chunks 0..3 only
    wiht = np.ascontiguousarray(
        W_ih[perm[: 4 * 128]]
        .T.reshape(NI, 128, 4, 128)
        .transpose(1, 2, 0, 3)
        .reshape(128, -1)
    ).astype(bf)
    # W8S [128, (c, j, k, h)]: fp8 weights for fused chunks g in [GFUSE, 16)
    w8s = np.ascontiguousarray(
        W_ih[perm[GFUSE * 128 :]]
        .reshape(NCH - GFUSE, 128, 2, 2, 128)  # [c, h, j, k, p]
        .transpose(4, 0, 2, 3, 1)
        .reshape(128, -1)
    ).astype(f8)
    # WHOT [128, NCH*O]: line p = [g, o] with value W_ho[o, h=g*128+p]
    whot = np.ascontiguousarray(
        W_ho[:, perm].T.reshape(NCH, 128, O).transpose(1, 0, 2).reshape(128, NCH * O)
    ).astype(bf)
    bias = np.tile(b_ho[None, :], (BC, 1)).astype(np.float32)

    common = {
        "WIHT": wiht,
        "W8S": w8s,
        "WHOT": whot,
        "BIAS": bias,
        "SCN": plan["SCN"].astype(bf),
    }
    if NF8:
        # W8 [128, (j,k,h)]: chunk-0 weights, i = (2j+k)*128 + p
        w0 = W_ih[perm[:128]]  # [128h, I]
        common["W8"] = np.ascontiguousarray(
            w0.T.reshape(2, 2, 128, 128).transpose(2, 0, 1, 3).reshape(128, 512)
        ).astype(f8)
    in_maps = []
    for m in range(NCORES):
        im = dict(common)
        xm = X[:, m * BC : (m + 1) * BC, :]  # [S, BC, I]
        # bf16 blocks: [128(i%128), (ic, b, tau)]
        xt = xm.transpose(2, 1, 0).reshape(NI, 128, BC, NBLK, TB)
        xt = np.ascontiguousarray(xt.transpose(3, 1, 0, 2, 4)).reshape(
            NBLK, 128, NI * BC * TB
        )
        im["XB"] = xt[NF8:].astype(bf)
        # X8S [128, (j, k, b, tau)]: fp8 copy of the last WG steps (block 15
        # tail) feeding the fused small-chunk GEMMs
        im["X8S"] = np.ascontiguousarray(
            xm[S - WG :]
            .transpose(2, 1, 0)  # [i, b, tau]
            .reshape(2, 2, 128, BC, WG)  # [j, k, p, b, tau]
            .transpose(2, 0, 1, 3, 4)
            .reshape(128, 4 * BC * WG)
        ).astype(f8)
        if NF8:
            x8 = np.empty((NF8, 128, 4 * BC * TB), dtype=f8)
            for kb in range(NF8):
                blk = xm[kb * TB : (kb + 1) * TB]  # [TB, BC, I]
                a = blk.transpose(2, 1, 0).reshape(2, 2, 128, BC, TB)  # [j,k,p,b,t]
                x8[kb] = (
                    np.ascontiguousarray(a.transpose(2, 0, 1, 3, 4))
                    .reshape(128, 4 * BC * TB)
                    .astype(f8)
                )
            # pack block pairs along the line dim: [NF8//2, 128, 8*BC*TB]
            im["X8"] = np.ascontiguousarray(
                x8.reshape(NF8 // 2, 2, 128, 4 * BC * TB).transpose(0, 2, 1, 3)
            ).reshape(NF8 // 2, 128, 8 * BC * TB)
        in_maps.append(im)

    trace = bool(int(os.environ.get("DIAG_TRACE", "0")))
    if trace:
        trace = _ensure_ntff_hook()
    res = None
    for attempt in range(3):
        try:
            res = bass_utils.run_bass_kernel_spmd(
                nc,
                in_maps,
                core_ids=list(range(NCORES)),
                trace=trace,
                tmpdir=os.environ.get("DIAG_TRACE_DIR") or None,
            )
            break
        except Exception:
            if attempt == 2:
                raise
            trace = False  # retry without profiling
    if res.exec_time_ns is not None:
        kernel.last_exec_time_ns = res.exec_time_ns
        kernel.last_mean_exec_time_ns = res.mean_exec_time_ns
    Yfull = np.concatenate([r["Y"] for r in res.results], axis=0)
    return Yfull


kernel.last_exec_time_ns = None
kernel.last_mean_exec_time_ns = None


# revision 64
# speedup vs baseline: 1.0638x; 1.0030x over previous
"""Trainium2 Bass kernel for nn_Diagnet (S=1024, B=64, I=512, H=2048, O=512).

    u = einsum('sbi,hi->sbh', X, W_ih)
    h_t = |u_t + hh * h_{t-1}|   (scan over S, only final h needed)
    Y = h_final @ W_ho.T + b_ho

Strategy (8 NeuronCores, data-parallel over batch, BC=8 rows per core):

* H lanes are permuted so hh is sorted descending and split into 16
  chunks of 128.  A chunk whose largest decay a satisfies a^K < tol
  only needs the last K steps, so each chunk gets a window K_g and the
  GEMM + scan skip everything earlier.  Chunk 0 keeps the full 1024.
* The recurrence is a custom DVE instruction that folds a window in
  one go: m[t] = |m[t-1] - u[t]*scn[t]| with scn[t] = -a^(K-1-t)
  (prescale folds the decay into the stream; the minus sign turns
  ABSOLUTE_DIFF into abs-add).  h_final = last element.
* All 8 batch rows fold in ONE scan instruction per segment via a
  3-column header per row: a BIG separator pair scaled (-1, +1)
  absorbs and exactly zeroes the running state between rows, then a
  seed column (scn=-1) re-injects that row's carry from the previous
  segment (h >= 0 so |0 - s*(-1)| = s).  Seed values are copied
  between segments by a tiny DVE copy, keeping the serial chain on
  one engine.
* Chunk-0 blocks t<640 run in fp8 (e4m3) with DoubleRow perf mode
  (256-deep contraction, half the matmul passes and half the X bytes);
  late blocks and everything else stay bf16.  Decay weighting keeps
  the fp8 quantization error ~1.5% of max|Y| (gate 2e-2).
* Small chunks g>=4 also run in fp8 (their h decay fast; g1..g3 carry
  too much output mass and stay bf16): one fused DoubleRow GEMM per
  chunk off a tiny fp8 copy of the block-15 tail, one fused scan and
  one fused h-extract for all twelve.
* Two HWDGE queues stream the inputs: the scalar queue prefetches the
  small fp8 operands + first chunk-0 pair (it clears its preamble
  ~2us before sync), while the sync queue streams, in priority order:
  X15, SCN, remaining fp8 X pairs, bf16 W (chunks 0..3), W_ho hi half,
  bf16 X blocks 10..13, W_ho lo half (its projections run in the final
  scan chain's shadow).  The PE chases arrivals; the 16 output
  projections interleave into DMA slack instead of trailing at the end.
"""

import math
import os

from contextlib import ExitStack

import numpy as np

S, B, I, H, O = 1024, 64, 512, 2048, 512
NCORES = 8
BC = B // NCORES  # 8 batch rows per core
TB = 64  # X block granularity
WG = 32  # truncation-window granularity
NBLK = S // TB  # 16
NCH = H // 128  # 16 h-chunks
NI = I // 128  # 4 i-chunks
USMALL_W = 256  # max window (cols) for chunks g>=1
# chunk-0 scan segments as (first_block, n_blocks); full coverage of 0..15.
# Blocks 12/13 arrive last, so they get 1-block segments to shorten the
# serial scan tail; 14/15 (resident early for the small chunks) scan last.
SEGS = [(0, 2), (2, 2), (4, 2), (6, 2), (8, 2), (10, 2), (12, 1), (13, 3)]
GFUSE = 4  # small chunks g >= GFUSE share one u tile / one fused scan
FW = 32  # fused-chunk window (these chunks all truncate to one half-block)
HDR = 3  # per-row header cols: BIG sep (-1), BIG sep (+1), seed (-1)

_CACHE = {}


def _fp8_nblk():
    n = int(os.environ.get("DIAG_FP8BLK", "10"))
    assert n % 2 == 0 and 0 <= n <= (S - USMALL_W) // TB, n
    return n


def _seg_meta():
    """Per-segment (start_col, width) in per-row cols, and block->seg map."""
    seg_w = [HDR + nb * TB for _, nb in SEGS]
    seg_c = np.concatenate([[0], np.cumsum(seg_w)]).astype(int)
    seg_of = {}
    for s, (fb, nb) in enumerate(SEGS):
        for i in range(nb):
            seg_of[fb + i] = (s, i)
    return seg_w, seg_c, seg_of  # widths, col starts (len nseg+1), block map


def _register_scan_ops():
    """Fold op: m[t] = |m[t-1] - in0[t]*in1[t]|, zero-initialized."""
    import concourse.dve_ops as dve_ops
    from concourse.dve_spec import C0, Spec, Src0, Src1, Zero, scan, lower, AluOp
    from concourse.dve_uop import DveOpSpec

    have = {op.name: op for op in dve_ops.OPS}
    if "ABSDIFF_SCALE_SCAN_ANT" in have:
        return have["ABSDIFF_SCALE_SCAN_ANT"]

    def _ref(in0, in1, s0, s1, imm2):
        x = in0.astype(np.float32) * in1.astype(np.float32)
        out = np.empty_like(x)
        m = np.zeros(x.shape[0], np.float32)
        for t in range(x.shape[1]):
            m = np.abs(m - x[:, t])
            out[:, t] = m
        return out

    spec = Spec(
        body=scan(AluOp.ABSOLUTE_DIFF, Src0 * Src1, init=Zero),
        reference=_ref,
    )
    row = max(dve_ops._SUB_OPCODE_FOR_NAME.values()) + 1
    assert row < 0x20
    shas = {}
    for ver in ("v3", "v4"):
        s = DveOpSpec(
            name="ABSDIFF_SCALE_SCAN_ANT", opcode=row, uops=lower(spec, ver=ver),
            rd1_en=True,
        )
        shas[ver] = s.sha(ver)
    op = dve_ops.DveOp("ABSDIFF_SCALE_SCAN_ANT", spec, subdim=False, uops_sha=shas)
    dve_ops._SUB_OPCODE_FOR_NAME["ABSDIFF_SCALE_SCAN_ANT"] = row
    dve_ops.OPS.append(op)
    dve_ops.CUSTOM_DVE_SPECS["ABSDIFF_SCALE_SCAN_ANT"] = spec
    return op


def _windows(hh):
    ln = float(os.environ.get("DIAG_LN", "7.0"))
    a = np.maximum(np.abs(hh.astype(np.float64)), 1e-30)
    perm = np.argsort(-a, kind="stable")
    ag = a[perm].reshape(NCH, 128)
    windows = []
    for g in range(NCH):
        amax = ag[g, 0]
        if S * math.log(amax) >= -ln:
            kg = S
        else:
            kg = int(math.ceil(ln / math.log(1.0 / amax)))
        kg = min(S, max(WG, ((kg + WG - 1) // WG) * WG))
        windows.append(kg)
    assert windows[0] == S, windows
    assert all(windows[g] >= windows[g + 1] for g in range(NCH - 1)), windows
    assert all(k <= USMALL_W for k in windows[1:]), (windows, "raise USMALL_W")
    return perm, ag, tuple(windows)


def _small_offs(windows, base):
    """Start col of each small chunk's scn piece (g>=1), after chunk-0 base."""
    widths = [0] + [windows[g] + 2 for g in range(1, NCH)]
    return base + np.cumsum(widths).astype(int)  # index by g-1 ... use [g-1]


def _make_plan(hh):
    perm, ag, windows = _windows(hh)
    seg_w, seg_c, _ = _seg_meta()
    u0w = int(seg_c[-1])  # per-row cols of chunk-0 stream (1045)
    offs = np.concatenate(
        [[u0w], u0w + np.cumsum([windows[g] + 2 for g in range(1, NCH)])]
    ).astype(int)
    total = int(offs[-1])
    scn = np.zeros((128, total), dtype=np.float64)
    a0 = ag[0]
    col = 0
    for (fb, nb) in SEGS:
        scn[:, col] = -1.0
        scn[:, col + 1] = 1.0
        scn[:, col + 2] = -1.0
        t = np.arange(fb * TB, (fb + nb) * TB)
        scn[:, col + HDR : col + HDR + nb * TB] = -(
            a0[:, None] ** (S - 1 - t)[None, :]
        )
        col += HDR + nb * TB
    assert col == u0w
    for g in range(1, NCH):
        kg = windows[g]
        off = int(offs[g - 1])
        tau = np.arange(kg)
        scn[:, off : off + kg] = -(ag[g][:, None] ** (kg - 1 - tau)[None, :])
        scn[:, off + kg] = -1.0
        scn[:, off + kg + 1] = 1.0
    return {"perm": perm, "windows": windows, "offs": offs, "SCN": scn}


def _build(windows):
    import concourse.mybir as mybir
    import concourse.tile as tile
    from concourse import bacc
    from concourse.bass import ds

    SCAN_OP = _register_scan_ops()
    f32 = mybir.dt.float32
    bf16 = mybir.dt.bfloat16
    f8 = mybir.dt.float8e4
    DR = mybir.MatmulPerfMode.DoubleRow
    NF8 = _fp8_nblk()

    seg_w, seg_c, seg_of = _seg_meta()
    u0w = int(seg_c[-1])
    offs = np.concatenate(
        [[u0w], u0w + np.cumsum([windows[g] + 2 for g in range(1, NCH)])]
    ).astype(int)
    total_scn = int(offs[-1])
    nseg = len(SEGS)

    nc = bacc.Bacc("TRN2", target_bir_lowering=False, debug=False, num_devices=NCORES)
    XB = nc.dram_tensor(
        "XB", [NBLK - NF8, 128, NI * TB * BC], bf16, kind="ExternalInput"
    ).ap()  # blocks NF8..15, line [ic, b, tau]
    if NF8:
        # blocks 0..NF8-1 packed in pairs; per block the line is
        # [j, k, b, tau] with i = (2j+k)*128+p.  Pairing keeps 4KB DMA lines.
        X8 = nc.dram_tensor(
            "X8", [NF8 // 2, 128, 8 * TB * BC], f8, kind="ExternalInput"
        ).ap()
        W8 = nc.dram_tensor("W8", [128, 4 * 128], f8, kind="ExternalInput").ap()
    # fp8 operands for the fused small chunks: block-15 tail steps of X and
    # the g4..g15 weight rows, both in DoubleRow [j, k, ...] layout
    NFU_B = NCH - GFUSE
    X8S = nc.dram_tensor("X8S", [128, 4 * BC * WG], f8, kind="ExternalInput").ap()
    W8S = nc.dram_tensor("W8S", [128, NFU_B * 4 * 128], f8, kind="ExternalInput").ap()
    # bf16 input weights only needed for chunks 0..3 now
    WIHT = nc.dram_tensor("WIHT", [128, 4 * NI * 128], bf16, kind="ExternalInput").ap()
    WHOT = nc.dram_tensor("WHOT", [128, NCH * O], bf16, kind="ExternalInput").ap()
    SCN = nc.dram_tensor("SCN", [128, total_scn], bf16, kind="ExternalInput").ap()
    BIAS = nc.dram_tensor("BIAS", [BC, O], f32, kind="ExternalInput").ap()
    Y = nc.dram_tensor("Y", [BC, O], f32, kind="ExternalOutput").ap()

    with tile.TileContext(nc) as tc:
        with ExitStack() as ctx:
            consts = ctx.enter_context(tc.tile_pool(name="consts", bufs=1))
            xpool = ctx.enter_context(tc.tile_pool(name="xt", bufs=1))
            ubig = ctx.enter_context(tc.tile_pool(name="ubig", bufs=1))
            usmall = ctx.enter_context(tc.tile_pool(name="usmall", bufs=4))
            ypool = ctx.enter_context(tc.tile_pool(name="yout", bufs=1))
            gpool = ctx.enter_context(tc.tile_pool(name="gpsum", bufs=7, space="PSUM"))
            fpool = ctx.enter_context(tc.tile_pool(name="fpsum", bufs=1, space="PSUM"))

            wiht_t = consts.tile([128, 4 * NI * 128], bf16, tag="wiht", name="wiht_t")
            w8s_t = consts.tile([128, (NCH - GFUSE) * 4 * 128], f8, tag="w8s", name="w8s_t")
            x8s_t = consts.tile([128, 4 * BC * WG], f8, tag="x8s", name="x8s_t")
            whot_t = consts.tile([128, NCH * O], bf16, tag="whot", name="whot_t")
            scn_t = consts.tile([128, total_scn], bf16, tag="scn", name="scn_t")
            bias_t = ypool.tile([BC, O], f32, tag="bias", name="bias_t")
            h_all = consts.tile([128, NCH * BC], bf16, tag="hall", name="h_all")
            if NF8:
                w8_t = consts.tile([128, 4 * 128], f8, tag="w8", name="w8_t")
            # fp8 X packed two blocks per tile (4KB DMA lines for full rate)
            x8pts = [
                xpool.tile([128, 8 * TB * BC], f8, tag=f"x8p_{pi}", name=f"x8p_{pi}")[:]
                for pi in range(NF8 // 2)
            ]

            def x8_half(kb):  # [p, (j,k,b,tau)] slice for one block
                return x8pts[kb // 2][:, ds((kb % 2) * 4 * TB * BC, 4 * TB * BC)]

            xbts = {
                kb: xpool.tile(
                    [128, NI * TB * BC], bf16, tag=f"xb_{kb}", name=f"xb_{kb}"
                )[:]
                for kb in range(NF8, NBLK)
            }
            u0_t = ubig.tile([128, BC * u0w], f32, tag="u0", name="u0")
            # fused u tile for small chunks g in [GFUSE, 16): [p, (b, c, FW+2)].
            # b-major so the fused scan is rank-3: [p, b, (c t)] with the scn
            # stream broadcast over b; each (b, c) sub-stream is independent.
            NFU = NCH - GFUSE
            u12_t = ubig.tile(
                [128, BC * NFU * (FW + 2)], f32, tag="u12", name="u12"
            )

            def u12_4d():  # [p, b, c, t]
                return u12_t[:].rearrange(
                    "p (b c t) -> p b c t", b=BC, c=NFU
                )

            def u0_seg(s):  # [p, b, width_s]
                return u0_t[:, ds(int(seg_c[s]) * BC, BC * seg_w[s])].rearrange(
                    "p (b t) -> p b t", b=BC
                )

            # --- DMA stream (single HWDGE queue; order = priority).
            # fp8 chunk-0 blocks go first so the GEMM->scan chain chases the
            # stream from the start; small-chunk weights and X14/15 interleave
            # to fill PE slack; chunk-0-only bf16 blocks 10..13 stream last. ---
            def wp(g0, ng):
                return ds(g0 * NI * 128, ng * NI * 128)

            def xbd(kb):
                nc.sync.dma_start(xbts[kb], XB[kb - NF8])

            dma = nc.sync.dma_start
            assert NF8 == 10, "stream schedule is tuned for DIAG_FP8BLK=10"
            # Prefetch the small-chunk fp8 operands + first chunk-0 pair on
            # the scalar HWDGE queue: it clears its preamble ~2us before sync
            # and doubles early DMA concurrency through the ramp.  A single
            # in-flight DMA tops out well below the aggregate rate, so W8S is
            # split in two.
            nc.scalar.dma_start(w8s_t[:], W8S)
            nc.scalar.dma_start(x8s_t[:], X8S)
            nc.scalar.dma_start(w8_t[:], W8)
            nc.scalar.dma_start(x8pts[0], X8[0])
            xbd(15)
            dma(scn_t[:], SCN)
            dma(x8pts[1], X8[1])
            xbd(14)
            dma(x8pts[2], X8[2])
            dma(x8pts[3], X8[3])
            dma(wiht_t[:], WIHT)  # bf16 weights for chunks 0..3
            dma(x8pts[4], X8[4])
            dma(whot_t[:, ds(8 * O, 8 * O)], WHOT[:, ds(8 * O, 8 * O)])  # g8..15
            xbd(10)
            xbd(11)
            xbd(12)
            xbd(13)
            # lo half of W_ho streams last: its projections run in the shadow
            # of the final scan chain, while XB12/13 (which gate that chain)
            # arrive earlier
            dma(whot_t[:, ds(0, 8 * O)], WHOT[:, ds(0, 8 * O)])  # g0..7
            dma(bias_t[:], BIAS)

            # --- header memsets for the fused scans ---
            for s in range(nseg):
                nc.gpsimd.memset(u0_seg(s)[:, :, ds(0, 2)], 1.0e30)
            nc.gpsimd.memset(u0_seg(0)[:, :, ds(2, 1)], 0.0)

            # --- PE warm-up: the HAM clock gate lifts a fixed ~5us after the
            # first matmul, so start that timer as early and cheaply as
            # possible (bf16: 1 cycle/col; fp32 warms cost 4x and queue
            # ahead of real work on the in-order PE) ---
            warm = consts.tile([128, TB * BC], bf16, tag="warm", name="warm")
            nc.gpsimd.memset(warm[:], 0.0)
            wps = gpool.tile([128, TB * BC], f32, tag="gp", name="warm_ps")
            NWARM = 2
            for i in range(NWARM):
                nc.tensor.matmul(
                    wps[:], warm[:, ds(0, 128)], warm[:],
                    start=(i == 0), stop=(i == NWARM - 1),
                )
            nc.scalar.copy(warm[:], wps[:])

            # --- output projection bookkeeping ---
            psy = fpool.tile([BC, O], f32, tag="fy", name="psy")
            proj_pending = list(range(NCH - 1, 0, -1))  # g15..g1; g0 last
            proj_done = [0]

            def emit_proj_g(g):
                nc.tensor.matmul(
                    psy[:], h_all[:, ds(g * BC, BC)], whot_t[:, ds(g * O, O)],
                    start=(proj_done[0] == 0), stop=(proj_done[0] == NCH - 1),
                )
                proj_done[0] += 1

            def emit_projs(n):
                for _ in range(min(n, len(proj_pending))):
                    emit_proj_g(proj_pending.pop(0))

            # --- small chunks: one GEMM unit per g.  g >= GFUSE (window WG,
            # block 15 only) write into the shared u12 tile and are scanned /
            # extracted by ONE fused instruction each at the end. ---
            for g in range(GFUSE, NCH):
                assert windows[g] == FW, (g, windows)
            nc.gpsimd.memset(
                u12_t[:].rearrange("p (x t) -> p x t", t=FW + 2)[:, :, ds(FW, 2)],
                1.0e30,
            )

            def emit_small_gemm(g):
                kg = windows[g]
                st0 = S - kg
                fb = st0 // TB
                toff = st0 % TB
                if g >= GFUSE:
                    u3 = u12_4d()[:, :, g - GFUSE]  # [p, b, WG+2]
                else:
                    u_t = usmall.tile(
                        [128, BC * (USMALL_W + 2)], f32, tag="us", name=f"u_g{g}"
                    )
                    u3 = u_t[:, ds(0, BC * (kg + 2))].rearrange(
                        "p (b t) -> p b t", b=BC
                    )
                    nc.gpsimd.memset(u3[:, :, ds(kg, 2)], 1.0e30)
                blocks = list(range(fb, NBLK))

                def t0_of(kb, fb=fb, toff=toff):
                    return toff if kb == fb else 0

                ps = {
                    kb: gpool.tile(
                        [128, (TB - t0_of(kb)) * BC], f32, tag="gp", name=f"gp_{g}_{kb}"
                    )
                    for kb in blocks
                }
                for ic in range(NI):
                    for kb in blocks:
                        t0 = t0_of(kb)
                        rhs = xbts[kb][:, ds(ic * TB * BC, TB * BC)]
                        out_ap = ps[kb][:]
                        if t0:
                            rhs = rhs.rearrange("p (b t) -> p b t", b=BC)[
                                :, :, ds(t0, TB - t0)
                            ]
                            out_ap = out_ap.rearrange("p (b t) -> p b t", b=BC)
                        nc.tensor.matmul(
                            out_ap,
                            wiht_t[:, ds(g * NI * 128 + ic * 128, 128)],
                            rhs,
                            start=(ic == 0),
                            stop=(ic == NI - 1),
                        )
                for kb in blocks:
                    t0 = t0_of(kb)
                    pos = kb * TB - st0 if kb > fb else 0
                    nc.scalar.copy(
                        u3[:, :, ds(pos, TB - t0)],
                        ps[kb][:].rearrange("p (b t) -> p b t", b=BC),
                    )
                return u3

            def emit_small(g):  # unfused path: GEMM + its own scan + extract
                kg = windows[g]
                u3 = emit_small_gemm(g)
                scn_g = (
                    scn_t[:, ds(int(offs[g - 1]), kg + 2)]
                    .rearrange("p (o t) -> p o t", o=1)
                    .broadcast_to([128, BC, kg + 2])
                )
                nc.vector._custom_dve(SCAN_OP, out=u3, in0=u3, in1=scn_g)
                nc.vector.tensor_copy(h_all[:, ds(g * BC, BC)], u3[:, :, kg - 1])

            def emit_small_f8(g):
                # fp8 DoubleRow GEMM for one fused chunk: X = block-15 tail
                # steps (x8s), weights from w8s; two j-passes of k=256
                c = g - GFUSE
                ps = gpool.tile([128, WG * BC], f32, tag="gp", name=f"gp8_{g}")
                for j in range(2):
                    w_ap = w8s_t[
                        :, ds(c * 4 * 128 + j * 2 * 128, 2 * 128)
                    ].rearrange("p (k h) -> p k h", k=2)
                    rhs = x8s_t[:, ds(j * 2 * BC * WG, 2 * BC * WG)].rearrange(
                        "p (k n) -> p k n", k=2
                    )
                    nc.tensor.matmul(
                        ps[:], w_ap, rhs,
                        start=(j == 0), stop=(j == 1), perf_mode=DR,
                    )
                nc.scalar.copy(
                    u12_4d()[:, :, c, ds(0, WG)],
                    ps[:].rearrange("p (b t) -> p b t", b=BC),
                )

            def emit_fused_small_scan():
                u3 = u12_t[:].rearrange(
                    "p (b ct) -> p b ct", b=BC
                )  # [p, b, NFU*(FW+2)]
                scn12 = (
                    scn_t[:, ds(int(offs[GFUSE - 1]), NFU * (FW + 2))]
                    .rearrange("p (o t) -> p o t", o=1)
                    .broadcast_to([128, BC, NFU * (FW + 2)])
                )
                nc.vector._custom_dve(SCAN_OP, out=u3, in0=u3, in1=scn12)
                # h extract for all fused chunks at once: dst is (c, b)-major,
                # so read the (b, c, t)-tile with c outer via strides
                src = u12_t[:].rearrange(
                    "p (b c t) -> p c b t", b=BC, c=NFU
                )[:, :, :, FW - 1]
                nc.vector.tensor_copy(h_all[:, ds(GFUSE * BC, NFU * BC)], src)

            # --- chunk 0: GEMM chases the DMA stream; fused per-seg scans ---
            seg_left = {s: nb for s, (_, nb) in enumerate(SEGS)}
            next_scan = [0]

            def copy_c0(kb, ps_t):
                s, i = seg_of[kb]
                nc.scalar.copy(
                    u0_seg(s)[:, :, ds(HDR + i * TB, TB)],
                    ps_t[:].rearrange("p (b t) -> p b t", b=BC),
                )
                seg_left[s] -= 1

            def emit_ready_scans():
                while next_scan[0] < nseg and seg_left[next_scan[0]] == 0:
                    s = next_scan[0]
                    w = seg_w[s]
                    if s > 0:
                        nc.vector.tensor_copy(
                            u0_seg(s)[:, :, 2], u0_seg(s - 1)[:, :, seg_w[s - 1] - 1]
                        )
                    scn_s = (
                        scn_t[:, ds(int(seg_c[s]), w)]
                        .rearrange("p (o t) -> p o t", o=1)
                        .broadcast_to([128, BC, w])
                    )
                    u3s = u0_seg(s)
                    nc.vector._custom_dve(SCAN_OP, out=u3s, in0=u3s, in1=scn_s)
                    next_scan[0] += 1

            def emit_c0_bf16_run(blocks):
                pss = {
                    kb: gpool.tile([128, TB * BC], f32, tag="gp", name=f"c0_{kb}")
                    for kb in blocks
                }
                for ic in range(NI):
                    for kb in blocks:
                        nc.tensor.matmul(
                            pss[kb][:],
                            wiht_t[:, ds(ic * 128, 128)],
                            xbts[kb][:, ds(ic * TB * BC, TB * BC)],
                            start=(ic == 0),
                            stop=(ic == NI - 1),
                        )
                for kb in blocks:
                    copy_c0(kb, pss[kb])

            def emit_c0_fp8_pair(k0):
                pair = (k0, k0 + 1)
                pss = {
                    kb: gpool.tile([128, TB * BC], f32, tag="gp", name=f"c8_{kb}")
                    for kb in pair
                }
                for j in range(2):
                    w_ap = w8_t[:].rearrange("p (j k h) -> p j k h", j=2, k=2)[:, j]
                    for kb in pair:
                        rhs = x8_half(kb).rearrange("p (j k n) -> p j k n", j=2, k=2)[
                            :, j
                        ]
                        nc.tensor.matmul(
                            pss[kb][:], w_ap, rhs,
                            start=(j == 0), stop=(j == 1), perf_mode=DR,
                        )
                for kb in pair:
                    copy_c0(kb, pss[kb])

            # --- interleaved schedule: the tiny fp8 small-chunk operands
            # arrive first, so all 12 fused small GEMMs run during the fp8
            # chunk-0 stream ramp; chunk-0 pairs chase the stream; bf16
            # blocks 10..13 trail; projections ride in DMA/scan shadows. ---
            for g in range(NCH - 1, GFUSE - 1, -1):
                emit_small_f8(g)
            emit_fused_small_scan()  # one scan + one extract for g4..g15
            emit_c0_fp8_pair(0)
            emit_ready_scans()
            emit_c0_fp8_pair(2)
            emit_ready_scans()
            emit_c0_fp8_pair(4)
            emit_ready_scans()
            emit_small(3)
            emit_small(2)
            emit_c0_fp8_pair(6)
            emit_ready_scans()
            emit_small(1)
            emit_c0_fp8_pair(8)
            emit_ready_scans()
            emit_c0_bf16_run([14, 15])  # resident early; tail-seg prefill
            emit_projs(4)  # g15..g12 (WHOT hi half)
            emit_c0_bf16_run([10])
            emit_ready_scans()
            emit_projs(2)
            emit_c0_bf16_run([11])
            emit_ready_scans()
            emit_projs(2)
            emit_c0_bf16_run([12])
            emit_ready_scans()
            emit_c0_bf16_run([13])
            emit_ready_scans()
            # remaining g-projections run in the shadow of the final segment
            # scans, fed by the late-arriving lo half of W_ho
            emit_projs(len(proj_pending))
            assert next_scan[0] == nseg and not any(seg_left.values()), (
                next_scan, seg_left,
            )
            # chunk-0 h extract, then its projection closes the accumulation
            nc.vector.tensor_copy(
                h_all[:, ds(0, BC)], u0_seg(nseg - 1)[:, :, seg_w[nseg - 1] - 1]
            )
            emit_proj_g(0)
            assert proj_done[0] == NCH

            y_t = ypool.tile([BC, O], f32, tag="y", name="y_t")
            # bias-add and output DMA in halves: the first half's DMA issue
            # (on sync) overlaps the second half's bias add, whose DMA goes
            # out on the scalar queue -- shortens the serial output path
            ho = O // 2
            nc.vector.tensor_tensor(
                y_t[:, ds(0, ho)], psy[:, ds(0, ho)], bias_t[:, ds(0, ho)],
                mybir.AluOpType.add,
            )
            nc.sync.dma_start(Y[:, ds(0, ho)], y_t[:, ds(0, ho)])
            nc.vector.tensor_tensor(
                y_t[:, ds(ho, ho)], psy[:, ds(ho, ho)], bias_t[:, ds(ho, ho)],
                mybir.AluOpType.add,
            )
            nc.scalar.dma_start(Y[:, ds(ho, ho)], y_t[:, ds(ho, ho)])
    nc.compile()
    return nc


def _get_program(windows):
    key = (windows, os.environ.get("DIAG_LN"), os.environ.get("DIAG_FP8BLK"))
    if key not in _CACHE:
        _CACHE[key] = _build(windows)
    return _CACHE[key]


def _ensure_ntff_hook():
    """Provide antenv.axon_hooks (absent in this image) so trace=True works."""
    import sys
    import types

    if "antenv.axon_hooks" in sys.modules:
        return True
    try:
        import antenv

        mod = types.ModuleType("antenv.axon_hooks")
        mod._hook = None

        def set_axon_ntff_profile_hook(h):
            mod._hook = h

        def get_axon_ntff_profile_hook():
            return mod._hook

        mod.set_axon_ntff_profile_hook = set_axon_ntff_profile_hook
        mod.get_axon_ntff_profile_hook = get_axon_ntff_profile_hook
        sys.modules["antenv.axon_hooks"] = mod
        antenv.axon_hooks = mod

        from trn_agent_boot.trn_boot import _ntff_profile_via_ctypes

        hook = _ntff_profile_via_ctypes("/opt/axon/libaxon_pjrt.so")
        mod.set_axon_ntff_profile_hook(hook)
        return hook is not None
    except Exception:
        return False


def kernel(X, W_ih, hh, W_ho, b_ho):
    import ml_dtypes

    from concourse import bass_utils

    X = np.asarray(X, dtype=np.float32)
    W_ih = np.asarray(W_ih, dtype=np.float32)
    hh = np.asarray(hh, dtype=np.float32)
    W_ho = np.asarray(W_ho, dtype=np.float32)
    b_ho = np.asarray(b_ho, dtype=np.float32)

    plan = _make_plan(hh)
    perm = plan["perm"]
    nc = _get_program(plan["windows"])
    NF8 = _fp8_nblk()

    bf = ml_dtypes.bfloat16
    f8 = ml_dtypes.float8_e4m3
    # WIHT [128, 4*NI*128]: line p = [g, ic, hsub] for chunks 0..3 only
    wiht = np.ascontiguousarray(
        W_ih[perm[: 4 * 128]]
        .T.reshape(NI, 128, 4, 128)
        .transpose(1, 2, 0, 3)
        .reshape(128, -1)
    ).astype(bf)
    # W8S [128, (c, j, k, h)]: fp8 weights for fused chunks g in [GFUSE, 16)
    w8s = np.ascontiguousarray(
        W_ih[perm[GFUSE * 128 :]]
        .reshape(NCH - GFUSE, 128, 2, 2, 128)  # [c, h, j, k, p]
        .transpose(4, 0, 2, 3, 1)
        .reshape(128, -1)
    ).astype(f8)
    # WHOT [128, NCH*O]: line p = [g, o] with value W_ho[o, h=g*128+p]
    whot = np.ascontiguousarray(
        W_ho[:, perm].T.reshape(NCH, 128, O).transpose(1, 0, 2).reshape(128, NCH * O)
    ).astype(bf)
    bias = np.tile(b_ho[None, :], (BC, 1)).astype(np.float32)

    common = {
        "WIHT": wiht,
        "W8S": w8s,
        "WHOT": whot,
        "BIAS": bias,
        "SCN": plan["SCN"].astype(bf),
    }
    if NF8:
        # W8 [128, (j,k,h)]: chunk-0 weights, i = (2j+k)*128 + p
        w0 = W_ih[perm[:128]]  # [128h, I]
        common["W8"] = np.ascontiguousarray(
            w0.T.reshape(2, 2, 128, 128).transpose(2, 0, 1, 3).reshape(128, 512)
        ).astype(f8)
    in_maps = []
    for m in range(NCORES):
        im = dict(common)
        xm = X[:, m * BC : (m + 1) * BC, :]  # [S, BC, I]
        # bf16 blocks: [128(i%128), (ic, b, tau)]
        xt = xm.transpose(2, 1, 0).reshape(NI, 128, BC, NBLK, TB)
        xt = np.ascontiguousarray(xt.transpose(3, 1, 0, 2, 4)).reshape(
            NBLK, 128, NI * BC * TB
        )
        im["XB"] = xt[NF8:].astype(bf)
        # X8S [128, (j, k, b, tau)]: fp8 copy of the last WG steps (block 15
        # tail) feeding the fused small-chunk GEMMs
        im["X8S"] = np.ascontiguousarray(
            xm[S - WG :]
            .transpose(2, 1, 0)  # [i, b, tau]
            .reshape(2, 2, 128, BC, WG)  # [j, k, p, b, tau]
            .transpose(2, 0, 1, 3, 4)
            .reshape(128, 4 * BC * WG)
        ).astype(f8)
        if NF8:
            x8 = np.empty((NF8, 128, 4 * BC * TB), dtype=f8)
            for kb in range(NF8):
                blk = xm[kb * TB : (kb + 1) * TB]  # [TB, BC, I]
                a = blk.transpose(2, 1, 0).reshape(2, 2, 128, BC, TB)  # [j,k,p,b,t]
                x8[kb] = (
                    np.ascontiguousarray(a.transpose(2, 0, 1, 3, 4))
                    .reshape(128, 4 * BC * TB)
                    .astype(f8)
                )
            # pack block pairs along the line dim: [NF8//2, 128, 8*BC*TB]
            im["X8"] = np.ascontiguousarray(
                x8.reshape(NF8 // 2, 2, 128, 4 * BC * TB).transpose(0, 2, 1, 3)
            ).reshape(NF8 // 2, 128, 8 * BC * TB)
        in_maps.append(im)

    trace = bool(int(os.environ.get("DIAG_TRACE", "0")))
    if trace:
        trace = _ensure_ntff_hook()
    res = None
    for attempt in range(3):
        try:
            res = bass_utils.run_bass_kernel_spmd(
                nc,
                in_maps,
                core_ids=list(range(NCORES)),
                trace=trace,
                tmpdir=os.environ.get("DIAG_TRACE_DIR") or None,
            )
            break
        except Exception:
            if attempt == 2:
                raise
            trace = False  # retry without profiling
    if res.exec_time_ns is not None:
        kernel.last_exec_time_ns = res.exec_time_ns
        kernel.last_mean_exec_time_ns = res.mean_exec_time_ns
    Yfull = np.concatenate([r["Y"] for r in res.results], axis=0)
    return Yfull


kernel.last_exec_time_ns = None
kernel.last_mean_exec_time_ns = None
